# revision 1
# baseline (speedup 1.0000x reference)
"""Trainium2 Bass kernel for nn_Decoder (3-step LSTM decoder w/ Luong attention
+ conv1d entity heads). Data-parallel over batch: B=64 -> 8 cores x 8.

Decomposition (validated vs reference to 5e-7):
  - conv1d over feat=[enc, broadcast(o)] splits into a 3-tap matmul conv over
    enc (shared by both ent_heads calls) plus a per-batch bias vec@Kvec (with
    first/last-column variants for the SAME-padding edges).
  - attend(q) = tanh(mix @ Wa[:, :E].T + q @ Wa[:, E:].T + b) with
    mix = softmax(q.enc) @ enc.
All heavy matmuls run in bf16 (fp32 PSUM accumulation).
"""
import numpy as np
import ml_dtypes
from contextlib import ExitStack

import concourse.bass as bass
import concourse.bacc as bacc
import concourse.tile as tile
from concourse import mybir
from concourse.bass_utils import run_bass_kernel_spmd
from concourse.masks import make_identity

B, S, E, R = 64, 2048, 256, 50
NCORES = 8
BC = B // NCORES          # batch per core = 8
NCH = S // 512            # 4 s-chunks of 512
F32 = mybir.dt.float32
BF16 = mybir.dt.bfloat16
Relu = mybir.ActivationFunctionType.Relu
Tanh = mybir.ActivationFunctionType.Tanh
Exp = mybir.ActivationFunctionType.Exp
OC = [R, R + S, R + 2 * S, R + 3 * S]   # output col starts: e1a,e2a,e1b,e2b

# packed bf16 weight blob layout: name -> (col offset, n cols) in [128, WTOT].
# Row-0-only entries (biases) still reserve full columns.
_WLAYOUT = [("Kenc", 1536), ("W_ihT", 2048), ("W_hhT", 2048), ("xT", 48),
            ("h0T", 16), ("bias_g", 1024), ("Wa_mT", 512), ("Wa_qT", 512),
            ("Kv_i", 512), ("Kv_f", 512), ("Kv_l", 512),
            ("W_relT", 2 * R), ("Went", 4),
            ("b_attn", 256), ("b_conv", 256), ("b_rel", R)]
WCRIT = 1536   # Kenc lands in its own first DMA so conv can start early
WOFF = {}
_o = 0
for _n, _c in _WLAYOUT:
    WOFF[_n] = (_o, _c)
    _o += _c
WTOT = _o


def _emit(ctx, tc, nc, io):
    P = 128
    wp = ctx.enter_context(tc.tile_pool(name="wp", bufs=1))
    ep = ctx.enter_context(tc.tile_pool(name="ep", bufs=1))
    sp = ctx.enter_context(tc.tile_pool(name="sp", bufs=2))
    bigp = ctx.enter_context(tc.tile_pool(name="bigp", bufs=1))
    rp = ctx.enter_context(tc.tile_pool(name="rp", bufs=4))
    psc = ctx.enter_context(tc.tile_pool(name="psc", bufs=3, space="PSUM"))
    pcv = ctx.enter_context(tc.tile_pool(name="pcv", bufs=2, space="PSUM"))
    psm = ctx.enter_context(tc.tile_pool(name="psm", bufs=3, space="PSUM"))
    
    dma = nc.sync.dma_start

    # ---- weights / constants: one packed bf16 blob, ONE DMA ----
    wsb = wp.tile([P, WTOT], BF16, name="wblob")
    dma(out=wsb[:, 0:WCRIT], in_=io["wblob"].ap()[:, 0:WCRIT])

    def wview(name, *dims):
        o, n = WOFF[name]
        v = wsb[:, o:o + n]
        if not dims:
            return v
        pat = "p (" + " ".join(f"d{i}" for i in range(len(dims) + 1)) + ") -> p " \
            + " ".join(f"d{i}" for i in range(len(dims) + 1))
        return v.rearrange(pat, **{f"d{i}": d for i, d in enumerate(dims)})

    def brow(name):
        o, n = WOFF[name]
        return wsb[0:1, o:o + n]

    W_ihT = wview("W_ihT", 2)
    W_hhT = wview("W_hhT", 2)
    Wa_mT = wview("Wa_mT", 2)
    Wa_qT = wview("Wa_qT", 2)
    Kv_i = wview("Kv_i", 2)
    Kv_f = wview("Kv_f", 2)
    Kv_l = wview("Kv_l", 2)
    Kenc = wview("Kenc", 3, 2, 2)
    W_relT = wview("W_relT", 2)
    Went = wview("Went", 2)
    xT = wview("xT", 3, 2)
    h0T = wview("h0T", 2)
    bias_g = brow("bias_g")
    b_attn = brow("b_attn")
    b_conv = brow("b_conv")
    b_rel = brow("b_rel")
    bent = wp.tile([2, 1], F32, name="bent")
    dma(out=bent[:], in_=io["bent"].ap())
    c0 = wp.tile([BC, E], F32, name="c0")
    dma(out=c0[:], in_=io["c0"].ap())

    ones_bf = wp.tile([1, BC], BF16, name="ones_bf")
    nc.vector.memset(ones_bf[:], 1.0)
    id_bf = wp.tile([P, P], BF16, name="id_bf")
    make_identity(nc, id_bf[:])
    id_f32 = wp.tile([P, P], F32, name="id_f32")
    make_identity(nc, id_f32[:])

    # ---- encoder tiles (both layouts, bf16, all 8 batches resident) ----
    encT = []   # [c(2x128 part), s] layout
    encS = []   # [s(16x128 part), c] layout
    for b in range(BC):
        tcs = bigp.tile([P, 2, S], BF16, name=f"encT{b}")
        for ch in range(2):
            dma(out=tcs[:, ch, :], in_=io["enc_cs"].ap()[b, ch * P:(ch + 1) * P, :])
        encT.append(tcs)
        if b == 0:
            dma(out=wsb[:, WCRIT:], in_=io["wblob"].ap()[:, WCRIT:])
    for b in range(BC):
        tsc = bigp.tile([P, 16, E], BF16, name=f"encS{b}")
        dma(out=tsc[:], in_=io["enc_sc"].ap()[b])
        encS.append(tsc)

    out_ap = io["out"].ap()

    # conv matmuls for one (batch, s-chunk, out-half) -> [128,512] psum
    def conv_half(b, j, half):
        s0 = j * 512
        ps = pcv.tile([P, 512], F32, name="conv_ps")
        first = True
        # center tap (w=1) first: always full width, so the start=True
        # matmul initializes every psum element before partial taps add
        for w in (1, 0, 2):
            lo = s0 + w - 1
            ob, oe = 0, 512
            if lo < 0:
                ob, lo = 1, 0
            elif lo + 512 > S:
                oe = 511
            for ch in range(2):
                nc.tensor.matmul(ps[:, ob:oe], Kenc[:, w, ch, half, :],
                                 encT[b][:, ch, lo:lo + (oe - ob)],
                                 start=first, stop=(w == 2 and ch == 1))
                first = False
        return ps


    # stage conv psum -> SBUF bf16 immediately: frees the psum slot without
    # waiting for the (late) vbias-gated relus, so conv streams continuously.
    # Pool depth throttles how far conv runs ahead of the relu consumers.
    stp = ctx.enter_context(tc.tile_pool(name="stp", bufs=27))

    def conv_stage(b, j, half):
        ps = conv_half(b, j, half)
        st = stp.tile([P, 512], BF16, name="cvst")
        nc.scalar.copy(st[:], ps[:])
        return st

    # ---- helper: transpose [BC, 2*P] sbuf -> [P, 2, BC] sbuf ----
    def transpose_to(src, dt, idt, name):
        dst = ep.tile([P, 2, BC], dt, name=name, bufs=2)
        for ch in range(2):
            pt = psm.tile([P, BC], dt, name="pt_tr", tag="ps")
            nc.tensor.transpose(pt[:], src[:, ch * P:(ch + 1) * P], idt[:BC, :BC])
            nc.scalar.copy(dst[:, ch, :], pt[:])
        return dst

    # ---- LSTM steps (batched over BC on partitions) ----
    # gates computed in two sequential [BC,512] halves through a single-bank
    # psum slot so the pool stays 1 bank (frees banks for conv/psm pipelines)
    def gates_half(t, hT, nch):
        gh = psm.tile([BC, 512], F32, name="gates", tag="ps")
        first = True
        for kh in range(2):
            nc.tensor.matmul(gh[:], xT[:, t, kh, :],
                             W_ihT[:, kh, nch * 512:(nch + 1) * 512],
                             start=first, stop=False); first = False
            nc.tensor.matmul(gh[:], hT[:, kh, :],
                             W_hhT[:, kh, nch * 512:(nch + 1) * 512],
                             start=False, stop=False)
        nc.tensor.matmul(gh[:], ones_bf[:], bias_g[:, nch * 512:(nch + 1) * 512],
                         start=False, stop=True)
        return gh

    def lstm_step(t, hT, c_prev):
        # i,f,g,o slices; sigmoid via tanh: sig(x)=0.5*tanh(x/2)+0.5
        g0 = gates_half(t, hT, 0)
        s_if = ep.tile([BC, 512], F32, name="s_if", bufs=1)
        nc.scalar.activation(s_if[:], g0[:], Tanh, scale=0.5)
        nc.vector.tensor_scalar(s_if[:], s_if[:], 0.5, 0.5,
                                op0=mybir.AluOpType.mult, op1=mybir.AluOpType.add)
        g1 = gates_half(t, hT, 1)
        t_g = ep.tile([BC, E], F32, name="t_g", bufs=1)
        nc.scalar.activation(t_g[:], g1[:, 0:256], Tanh)
        s_o = ep.tile([BC, E], F32, name="s_o", bufs=1)
        nc.scalar.activation(s_o[:], g1[:, 256:512], Tanh, scale=0.5)
        nc.vector.tensor_scalar(s_o[:], s_o[:], 0.5, 0.5,
                                op0=mybir.AluOpType.mult, op1=mybir.AluOpType.add)
        c2 = ep.tile([BC, E], F32, name="c2", bufs=2)
        nc.vector.tensor_mul(c2[:], s_if[:, 256:512], c_prev[:])
        tmp = ep.tile([BC, E], F32, name="tmp_ig", bufs=1)
        nc.vector.tensor_mul(tmp[:], s_if[:, 0:256], t_g[:])
        nc.vector.tensor_add(c2[:], c2[:], tmp[:])
        tc2 = ep.tile([BC, E], F32, name="tc2", bufs=1)
        nc.scalar.activation(tc2[:], c2[:], Tanh)
        h2 = ep.tile([BC, E], BF16, name="h2", bufs=2)
        nc.vector.tensor_mul(h2[:], s_o[:], tc2[:])
        h2T = transpose_to(h2, BF16, id_bf, f"h2T_{t}")
        return h2, h2T, c2

    # ---- attention setup: all 3 attends (q = h1, h2, h3) batched ----
    # row index r = a*BC + b (a = attend/step, b = batch). One sweep over the
    # encoder serves all three queries: 3x less PE streaming than per-attend.
    # qTm columns are filled right after each LSTM step (off the scores path).
    NQ = 3 * BC  # 24
    qTm = sp.tile([P, 2, BC, NQ], BF16, name="qTm", bufs=1)
    nc.vector.memset(qTm[:], 0.0)

    def fill_qTm(a, hT):
        for ch in range(2):
            for b in range(BC):
                nc.vector.tensor_copy(qTm[:, ch, b, a * BC + b:a * BC + b + 1],
                                      hT[:, ch, b:b + 1])

    h1, h1T, c1 = lstm_step(0, h0T, c0)
    fill_qTm(0, h1T)
    h2, h2T, c2 = lstm_step(1, h1T, c1)
    fill_qTm(1, h2T)
    h3, h3T, c3 = lstm_step(2, h2T, c2)
    fill_qTm(2, h3T)

    att = sp.tile([NQ, S], BF16, name="att", bufs=1)
    pexp = ep.tile([NQ, NCH], F32, name="pexp")
    for j in range(NCH):
        sps = psc.tile([NQ, 512], F32, name="sc_ps", tag="seb")
        for b in range(BC):
            for ch in range(2):
                nc.tensor.matmul(sps[:], qTm[:, ch, b, :],
                                 encT[b][:, ch, j * 512:(j + 1) * 512],
                                 start=(b == 0 and ch == 0),
                                 stop=(b == BC - 1 and ch == 1))
        # scores are bounded (|s| ~ 30 << 88): unshifted fp32 exp can't
        # overflow, and reading the psum chunk directly skips an sbuf copy
        nc.scalar.activation(att[:, j * 512:(j + 1) * 512], sps[:], Exp,
                             accum_out=pexp[:, j:j + 1])
    sm = ep.tile([NQ, 1], F32, name="sm")
    nc.vector.reduce_sum(sm[:], pexp[:], axis=mybir.AxisListType.X)
    rs = ep.tile([NQ, 1], F32, name="rs")
    nc.vector.reciprocal(rs[:], sm[:])
    nc.vector.tensor_scalar_mul(att[:], att[:], rs[:])
    # transpose attn to [s-partition] tiles; one tile per j so mix matmuls
    # can start as soon as the first transpose lands
    attT = []
    for j in range(16):
        pt = psm.tile([P, NQ], BF16, name="pt_at", tag="ps")
        nc.tensor.transpose(pt[:], att[:, j * P:(j + 1) * P], id_bf[:NQ, :NQ])
        aj = sp.tile([P, NQ], BF16, name=f"attT{j}", bufs=1)
        nc.vector.tensor_copy(aj[:], pt[:])
        attT.append(aj)
    # mix: one [NQ, E] accumulation per b; rows {b, BC+b, 2*BC+b} are valid.
    # Engines can't address partition offsets, so copy the full tile,
    # PE-transpose it, and pick columns (free-dim offsets).
    mixTs = [ep.tile([P, 2, BC], BF16, name=f"mixT_t{a + 1}", bufs=2)
             for a in range(3)]
    for b in range(BC):
        mps = psm.tile([NQ, E], F32, name="mix_ps", tag="ps")
        for j in range(16):
            nc.tensor.matmul(mps[:], attT[j][:], encS[b][:, j, :],
                             start=(j == 0), stop=(j == 15))
        mfull = ep.tile([NQ, E], BF16, name="mfull", bufs=2)
        nc.scalar.copy(mfull[:], mps[:])
        for ch in range(2):
            pt = psm.tile([P, NQ], BF16, name="pt_mx", tag="ps")
            nc.tensor.transpose(pt[:], mfull[:, ch * P:(ch + 1) * P],
                                id_bf[:NQ, :NQ])
            for a in range(3):
                nc.vector.tensor_copy(mixTs[a][:, ch, b:b + 1],
                                      pt[:, a * BC + b:a * BC + b + 1])

    def attend_out(mixT, qT, tag):
        aps = psm.tile([BC, E], F32, name="ao_ps", tag="ps")
        for ch in range(2):
            nc.tensor.matmul(aps[:], mixT[:, ch, :], Wa_mT[:, ch, :],
                             start=(ch == 0), stop=False)
        for ch in range(2):
            nc.tensor.matmul(aps[:], qT[:, ch, :], Wa_qT[:, ch, :],
                             start=False, stop=False)
        nc.tensor.matmul(aps[:], ones_bf[:], b_attn[:], start=False, stop=True)
        o = ep.tile([BC, E], BF16, name=f"out_{tag}", bufs=1)
        nc.scalar.activation(o[:], aps[:], Tanh)
        oT = transpose_to(o, BF16, id_bf, f"outT_{tag}")
        return o, oT

    out2, out2T = attend_out(mixTs[1], h2T, "t2")
    out3, out3T = attend_out(mixTs[2], h3T, "t3")
    out1, out1T = attend_out(mixTs[0], h1T, "t1")

    # t1_out = out1 @ W_rel.T + b_rel -> out[:, 0:R]
    t1ps = psm.tile([BC, R], F32, name="t1_ps", tag="ps")
    for ch in range(2):
        nc.tensor.matmul(t1ps[:], out1T[:, ch, :], W_relT[:, ch, :],
                         start=(ch == 0), stop=False)
    nc.tensor.matmul(t1ps[:], ones_bf[:], b_rel[:], start=False, stop=True)
    t1sb = ep.tile([BC, R], F32, name="t1sb")
    nc.scalar.copy(t1sb[:], t1ps[:])
    dma(out=out_ap[:, 0:R], in_=t1sb[:])

    # ---- vbias variants: vb = o @ Kv_x + b_conv, transposed to [P,2,BC] ----
    def vbias(oT, Kv, tag):
        vps = psm.tile([BC, E], F32, name="vb_ps", tag="ps")
        for ch in range(2):
            nc.tensor.matmul(vps[:], oT[:, ch, :], Kv[:, ch, :],
                             start=(ch == 0), stop=False)
        nc.tensor.matmul(vps[:], ones_bf[:], b_conv[:], start=False, stop=True)
        vsb = ep.tile([BC, E], F32, name="vb_sb", bufs=2)
        nc.vector.tensor_copy(vsb[:], vps[:])
        return transpose_to(vsb, F32, id_f32, f"vbT_{tag}")

    vbA = [vbias(out2T, kv, f"a{i}") for i, kv in enumerate((Kv_i, Kv_f, Kv_l))]
    vbB = [vbias(out3T, kv, f"b{i}") for i, kv in enumerate((Kv_i, Kv_f, Kv_l))]

    # ---- conv + relu + entity-head reduction ----
    for b in range(BC):
        for j in range(NCH):
            s0 = j * 512
            cps = [conv_stage(b, j, half) for half in range(2)]
            for v, vbs in enumerate((vbA, vbB)):
                ent_ps = psc.tile([2, 512], F32, name="ent_ps", tag="seb")
                for half in range(2):
                    r = rp.tile([P, 512], BF16, name="relu")
                    nc.vector.tensor_scalar(r[:], cps[half][:],
                                            vbs[0][:, half, b:b + 1], 0.0,
                                            op0=mybir.AluOpType.add,
                                            op1=mybir.AluOpType.max)
                    if j == 0:
                        nc.vector.tensor_scalar(r[:, 0:1], cps[half][:, 0:1],
                                                vbs[1][:, half, b:b + 1], 0.0,
                                                op0=mybir.AluOpType.add,
                                                op1=mybir.AluOpType.max)
                    if j == NCH - 1:
                        nc.vector.tensor_scalar(r[:, 511:512], cps[half][:, 511:512],
                                                vbs[2][:, half, b:b + 1], 0.0,
                                                op0=mybir.AluOpType.add,
                                                op1=mybir.AluOpType.max)
                    nc.tensor.matmul(ent_ps[:], Went[:, half, :], r[:],
                                     start=(half == 0), stop=(half == 1))
                esb = ep.tile([2, 512], F32, name="esb", bufs=3)
                nc.scalar.activation(esb[:], ent_ps[:],
                                     mybir.ActivationFunctionType.Identity,
                                     bias=bent[:])
                dma(out=out_ap[b:b + 1, OC[2 * v] + s0:OC[2 * v] + s0 + 512],
                    in_=esb[0:1, :])
                dma(out=out_ap[b:b + 1, OC[2 * v + 1] + s0:OC[2 * v + 1] + s0 + 512],
                    in_=esb[1:2, :])


def build_nc():
    nc = bacc.Bacc("TRN2", target_bir_lowering=False, debug=False)
    io = {}

    def din(name, shape, dt):
        io[name] = nc.dram_tensor(name, shape, dt, kind="ExternalInput")

    din("enc_cs", [BC, E, S], BF16)
    din("enc_sc", [BC, 128, 16, E], BF16)
    din("wblob", [128, WTOT], BF16)
    din("bent", [2, 1], F32)
    din("c0", [BC, E], F32)
    io["out"] = nc.dram_tensor("out", [BC, R + 4 * S], F32, kind="ExternalOutput")

    with ExitStack() as ctx:
        t = ctx.enter_context(tile.TileContext(nc))
        _emit(ctx, t, nc, io)
    nc.compile()
    return nc


def _pack2(w):  # [256, N] fp32 -> [128, 2, N]
    return np.ascontiguousarray(w.reshape(2, 128, -1).transpose(1, 0, 2))


def prepare_in_maps(inputs):
    bf = ml_dtypes.bfloat16
    enc = np.asarray(inputs["encoder_o"], np.float32)
    enc_bf = enc.astype(bf)
    enc_cs = np.ascontiguousarray(enc_bf.transpose(0, 2, 1))
    W_ih = np.asarray(inputs["W_ih"], np.float32)
    W_hh = np.asarray(inputs["W_hh"], np.float32)
    W_attn = np.asarray(inputs["W_attn"], np.float32)
    kern = np.asarray(inputs["W_conv"], np.float32).transpose(2, 1, 0)  # [3,2E,E]
    Kenc_ = kern[:, :E, :]
    Kv = kern[:, E:, :]
    Kv_i, Kv_f, Kv_l = Kv.sum(0), Kv[1] + Kv[2], Kv[0] + Kv[1]
    # Kenc pack [128, 3, 2, 2, 128]: [p,w,ch,half,m] = Kenc_[w, ch*128+p, half*128+m]
    kp = Kenc_.reshape(3, 2, 128, 2, 128).transpose(2, 0, 1, 3, 4)
    We = np.stack([np.asarray(inputs["W_ent1"])[0], np.asarray(inputs["W_ent2"])[0]], 1)
    x1 = np.broadcast_to(np.asarray(inputs["sos_emb"])[0], (B, E))
    x2 = np.asarray(inputs["rel_emb"])[np.asarray(inputs["r_in"]).astype(np.int64)]
    idx = np.arange(B)
    k1 = np.asarray(inputs["k1"])[:, 0].astype(np.int64)
    k2 = np.asarray(inputs["k2"])[:, 0].astype(np.int64)
    x3 = enc[idx, k1] + enc[idx, k2]
    X = np.stack([x1, x2, x3], 0).astype(np.float32)      # [3,B,E]
    h0 = np.asarray(inputs["h0"], np.float32)[0]
    c0 = np.asarray(inputs["c0"], np.float32)

    wsh = np.zeros((128, WTOT), np.float32)

    def put(name, arr):                      # arr -> [128, n] block
        o, n = WOFF[name]
        wsh[:, o:o + n] = arr.reshape(128, n)

    def putrow(name, vec):                   # row-0 bias entries
        o, n = WOFF[name]
        wsh[0, o:o + n] = vec.ravel()

    put("W_ihT", _pack2(W_ih.T))
    put("W_hhT", _pack2(W_hh.T))
    put("Wa_mT", _pack2(W_attn[:, :E].T))
    put("Wa_qT", _pack2(W_attn[:, E:].T))
    put("Kv_i", _pack2(Kv_i))
    put("Kv_f", _pack2(Kv_f))
    put("Kv_l", _pack2(Kv_l))
    put("Kenc", np.ascontiguousarray(kp))
    put("W_relT", _pack2(np.asarray(inputs["W_rel"], np.float32).T))
    put("Went", _pack2(We))
    putrow("bias_g", np.asarray(inputs["b_ih"], np.float32)
           + np.asarray(inputs["b_hh"], np.float32))
    putrow("b_attn", np.asarray(inputs["b_attn"], np.float32))
    putrow("b_conv", np.asarray(inputs["b_conv"], np.float32))
    putrow("b_rel", np.asarray(inputs["b_rel"], np.float32))
    bent = np.array([[np.asarray(inputs["b_ent1"]).ravel()[0]],
                     [np.asarray(inputs["b_ent2"]).ravel()[0]]], np.float32)
    in_maps = []
    for c in range(NCORES):
        sl = slice(c * BC, (c + 1) * BC)
        w = wsh.copy()
        xs = X[:, sl]                                      # [3,BC,E]
        xo, xn = WOFF["xT"]
        w[:, xo:xo + xn] = xs.transpose(2, 0, 1).reshape(
            2, 128, 3, BC).transpose(1, 2, 0, 3).reshape(128, xn)
        ho, hn = WOFF["h0T"]
        w[:, ho:ho + hn] = h0[sl].T.reshape(2, 128, BC).transpose(
            1, 0, 2).reshape(128, hn)
        m = {
            "enc_cs": np.ascontiguousarray(enc_cs[sl]),
            "enc_sc": np.ascontiguousarray(
                enc_bf[sl].reshape(BC, 16, 128, E).transpose(0, 2, 1, 3)),
            "wblob": w.astype(bf),
            "bent": bent,
            "c0": np.ascontiguousarray(c0[0, sl]) if c0.ndim == 3
            else np.ascontiguousarray(c0[sl]),
        }
        in_maps.append(m)
    return in_maps


_NC_CACHE = {}


def get_nc():
    if "nc" not in _NC_CACHE:
        _NC_CACHE["nc"] = build_nc()
    return _NC_CACHE["nc"]


def kernel(**inputs) -> np.ndarray:
    nc = get_nc()
    in_maps = prepare_in_maps(inputs)
    res = run_bass_kernel_spmd(nc, in_maps, core_ids=list(range(NCORES)))
    return np.concatenate([r["out"] for r in res.results], 0).astype(np.float32)


if __name__ == "__main__":
    import jax
    import reference as refmod
    with jax.default_device(jax.devices("cpu")[0]):
        inputs = {k: np.asarray(v) for k, v in refmod.setup_inputs().items()}
        expected = np.asarray(refmod.reference(**inputs))
    actual = kernel(**inputs)
    err = np.abs(actual - expected)
    print("max abs err:", err.max(), "rel:", err.max() / np.abs(expected).max())



# revision 13
# speedup vs baseline: 1.1707x; 1.1707x over previous
"""Trainium2 Bass kernel for nn_Decoder (3-step LSTM decoder w/ Luong attention
+ conv1d entity heads). Data-parallel over batch: B=64 -> 8 cores x 8.

Decomposition (validated vs reference):
  - conv1d over feat=[enc, broadcast(o)] splits into a 3-tap matmul conv over
    enc (shared by both ent_heads calls) plus a per-batch bias vec@Kvec (with
    first/last-column variants for the SAME-padding edges).
  - attend(q) = tanh(mix @ Wa[:, :E].T + q @ Wa[:, E:].T + b) with
    mix = softmax(q.enc) @ enc.
Precision strategy (numerically validated in numsim.py, ~5e-3 rel err):
  - conv over enc in fp8 DoubleRow with 3-term compensation:
      conv = K8.ehi + K8.elo + dK16.(enc/16),  ehi=fp8(enc), elo=fp8(enc-ehi)
  - scores in fp8 DoubleRow, 3-term: qh.ehi + qh.elo + ql.ehi
  - mix / gates / attend / vbias / ent-head matmuls in bf16 (fp32 PSUM).
Entity-head outputs for (b,j) pack 4 rows (e1a,e2a,e1b,e2b) in one PSUM tile
via zero-padded Went weights -> one Act copy + one DMA per (b,j).
"""
import numpy as np
import ml_dtypes
from contextlib import ExitStack

import concourse.bass as bass
import concourse.bacc as bacc
import concourse.tile as tile
from concourse import mybir
from concourse.bass_utils import run_bass_kernel_spmd
from concourse.masks import make_identity

B, S, E, R = 64, 2048, 256, 50
NCORES = 8
BC = B // NCORES          # batch per core = 8
NCH = S // 512            # 4 s-chunks of 512
NQ = 3 * BC               # 24 attention queries
F32 = mybir.dt.float32
BF16 = mybir.dt.bfloat16
F8 = mybir.dt.float8e4
DR = mybir.MatmulPerfMode.DoubleRow
Relu = mybir.ActivationFunctionType.Relu
Tanh = mybir.ActivationFunctionType.Tanh
Exp = mybir.ActivationFunctionType.Exp
Ident = mybir.ActivationFunctionType.Identity

# packed bf16 weight blob layout: name -> (col offset, n cols) in [128, WTOT].
# part 1 (cols < WCRIT) is the LSTM-critical prefix, DMA'd first.
_WLAYOUT = [("W_ihT", 2048), ("W_hhT", 2048), ("xT", 48), ("h0T", 16),
            ("bias_g", 1024),
            ("Wa_mT", 512), ("Wa_qT", 512),
            ("Kv_i", 512), ("Kv_f", 512), ("Kv_l", 512),
            ("W_relT", 2 * R), ("WentP", 16),
            ("b_attn", 256), ("b_conv", 256), ("b_rel", R)]
WCRIT = 2048 + 2048 + 48 + 16 + 1024     # 5184
WOFF = {}
_o = 0
for _n, _c in _WLAYOUT:
    WOFF[_n] = (_o, _c)
    _o += _c
WTOT = _o

# fp8 conv weights in DoubleRow layout: K8 (e4m3) + dK (e5m2, the
# K-quantization residual; small values sit in e5m2's normal range)
W8TOT = 1536


def _emit(ctx, tc, nc, io):
    P = 128
    wp = ctx.enter_context(tc.tile_pool(name="wp", bufs=1))
    ep = ctx.enter_context(tc.tile_pool(name="ep", bufs=1))
    sp = ctx.enter_context(tc.tile_pool(name="sp", bufs=2))
    bigp = ctx.enter_context(tc.tile_pool(name="bigp", bufs=1))
    encsp = ctx.enter_context(tc.tile_pool(name="encsp", bufs=3))
    stp = ctx.enter_context(tc.tile_pool(name="stp", bufs=56))
    rp = ctx.enter_context(tc.tile_pool(name="rp", bufs=4))
    psc = ctx.enter_context(tc.tile_pool(name="psc", bufs=2, space="PSUM"))
    pcv = ctx.enter_context(tc.tile_pool(name="pcv", bufs=2, space="PSUM"))
    psm = ctx.enter_context(tc.tile_pool(name="psm", bufs=2, space="PSUM"))
    pent = ctx.enter_context(tc.tile_pool(name="pent", bufs=2, space="PSUM"))

    dma = nc.sync.dma_start

    # ---- weights / constants ----
    wsb = wp.tile([P, WTOT], BF16, name="wblob")
    w8sb = wp.tile([P, W8TOT], F8, name="wblob8")
    w85sb = wp.tile([P, W8TOT], mybir.dt.float8e5, name="wblob85")
    bent4 = wp.tile([4, 1], F32, name="bent4")
    c0 = wp.tile([BC, E], F32, name="c0")
    dma(out=wsb[:, 0:WCRIT], in_=io["wblob"].ap()[:, 0:WCRIT])
    dma(out=bent4[:], in_=io["bent4"].ap())
    dma(out=c0[:], in_=io["c0"].ap())
    dma(out=w8sb[:], in_=io["wblob8"].ap())
    dma(out=w85sb[:], in_=io["wblob85"].ap())

    # ---- enc fp8 (ehi, elo) resident tiles; e16/encS pooled with their DMAs
    # emitted next to their consumers (separate queues) so pool reuse sees
    # the readers.
    ehl = []
    for b in range(BC):
        t = bigp.tile([P, 2, 2, S], F8, name=f"ehl{b}")
        dma(out=t[:], in_=io["ehl"].ap()[b])
        ehl.append(t)
        if b == 2:
            dma(out=wsb[:, WCRIT:], in_=io["wblob"].ap()[:, WCRIT:])

    def wview(name, *dims):
        o, n = WOFF[name]
        v = wsb[:, o:o + n]
        if not dims:
            return v
        pat = "p (" + " ".join(f"d{i}" for i in range(len(dims) + 1)) + ") -> p " \
            + " ".join(f"d{i}" for i in range(len(dims) + 1))
        return v.rearrange(pat, **{f"d{i}": d for i, d in enumerate(dims)})

    def w8view(sb, *dims):
        pat = "p (" + " ".join(f"d{i}" for i in range(len(dims) + 1)) + ") -> p " \
            + " ".join(f"d{i}" for i in range(len(dims) + 1))
        return sb[:, :].rearrange(pat, **{f"d{i}": d for i, d in enumerate(dims)})

    def brow(name):
        o, n = WOFF[name]
        return wsb[0:1, o:o + n]

    W_ihT = wview("W_ihT", 2)
    W_hhT = wview("W_hhT", 2)
    Wa_mT = wview("Wa_mT", 2)
    Wa_qT = wview("Wa_qT", 2)
    Kv_i = wview("Kv_i", 2)
    Kv_f = wview("Kv_f", 2)
    Kv_l = wview("Kv_l", 2)
    W_relT = wview("W_relT", 2)
    WentP = wview("WentP", 2, 2)       # [128, v2, half2, 4]
    xT = wview("xT", 3, 2)
    h0T = wview("h0T", 2)
    bias_g = brow("bias_g")
    b_attn = brow("b_attn")
    b_conv = brow("b_conv")
    b_rel = brow("b_rel")
    K8 = w8view(w8sb, 3, 2, 2)         # [128, w, co_half, ci_t, 128]
    dK5 = w8view(w85sb, 3, 2, 2)

    ones_bf = wp.tile([1, BC], BF16, name="ones_bf")
    nc.vector.memset(ones_bf[:], 1.0)
    id_bf = wp.tile([P, P], BF16, name="id_bf")
    make_identity(nc, id_bf[:])
    id_f32 = wp.tile([P, P], F32, name="id_f32")
    make_identity(nc, id_f32[:])

    out_ap = io["out"].ap()
    outv = out_ap[:, R:].rearrange("b (vh s) -> b vh s", vh=4)

    # ---- helper: transpose [BC, 2*P] sbuf -> [P, 2, BC] sbuf ----
    def transpose_to(src, dt, idt, name):
        dst = ep.tile([P, 2, BC], dt, name=name, bufs=2)
        for ch in range(2):
            pt = psm.tile([P, BC], dt, name="pt_tr", tag="ps")
            nc.tensor.transpose(pt[:], src[:, ch * P:(ch + 1) * P], idt[:BC, :BC])
            nc.scalar.copy(dst[:, ch, :], pt[:])
        return dst

    # ---- LSTM steps (batched over BC on partitions) ----
    def gates_half(t, hT, nch):
        gh = psm.tile([BC, 512], F32, name="gates", tag="ps")
        first = True
        for kh in range(2):
            nc.tensor.matmul(gh[:], xT[:, t, kh, :],
                             W_ihT[:, kh, nch * 512:(nch + 1) * 512],
                             start=first, stop=False); first = False
            nc.tensor.matmul(gh[:], hT[:, kh, :],
                             W_hhT[:, kh, nch * 512:(nch + 1) * 512],
                             start=False, stop=False)
        nc.tensor.matmul(gh[:], ones_bf[:], bias_g[:, nch * 512:(nch + 1) * 512],
                         start=False, stop=True)
        return gh

    def lstm_step(t, hT, c_prev):
        g0 = gates_half(t, hT, 0)
        s_if = ep.tile([BC, 512], F32, name="s_if", bufs=1)
        nc.scalar.activation(s_if[:], g0[:], Tanh, scale=0.5)
        nc.vector.tensor_scalar(s_if[:], s_if[:], 0.5, 0.5,
                                op0=mybir.AluOpType.mult, op1=mybir.AluOpType.add)
        g1 = gates_half(t, hT, 1)
        t_g = ep.tile([BC, E], F32, name="t_g", bufs=1)
        nc.scalar.activation(t_g[:], g1[:, 0:256], Tanh)
        s_o = ep.tile([BC, E], F32, name="s_o", bufs=1)
        nc.scalar.activation(s_o[:], g1[:, 256:512], Tanh, scale=0.5)
        nc.vector.tensor_scalar(s_o[:], s_o[:], 0.5, 0.5,
                                op0=mybir.AluOpType.mult, op1=mybir.AluOpType.add)
        c2 = ep.tile([BC, E], F32, name="c2", bufs=2)
        nc.vector.tensor_mul(c2[:], s_if[:, 256:512], c_prev[:])
        tmp = ep.tile([BC, E], F32, name="tmp_ig", bufs=1)
        nc.vector.tensor_mul(tmp[:], s_if[:, 0:256], t_g[:])
        nc.vector.tensor_add(c2[:], c2[:], tmp[:])
        tc2 = ep.tile([BC, E], F32, name="tc2", bufs=1)
        nc.scalar.activation(tc2[:], c2[:], Tanh)
        h2 = ep.tile([BC, E], BF16, name="h2", bufs=2)
        nc.vector.tensor_mul(h2[:], s_o[:], tc2[:])
        h2T = transpose_to(h2, BF16, id_bf, f"h2T_{t}")
        return h2, h2T, c2

    # ---- attention: all 3 attends (q = h1, h2, h3) batched.
    # qTm columns zero except (a*BC+b) for batch b -> scores accumulate over b.
    qTm = sp.tile([P, 2, BC, NQ], BF16, name="qTm", bufs=1)
    nc.vector.memset(qTm[:], 0.0)

    def fill_qTm(a, hT):
        for ch in range(2):
            for b in range(BC):
                nc.vector.tensor_copy(qTm[:, ch, b, a * BC + b:a * BC + b + 1],
                                      hT[:, ch, b:b + 1])

    h1, h1T, c1 = lstm_step(0, h0T, c0)
    fill_qTm(0, h1T)
    h2, h2T, c2 = lstm_step(1, h1T, c1)
    fill_qTm(1, h2T)
    h3, h3T, c3 = lstm_step(2, h2T, c2)
    fill_qTm(2, h3T)

    # fp8 hi/lo split of the queries for compensated fp8 scores
    q8h = sp.tile([P, 2, BC, NQ], F8, name="q8h", bufs=1)
    nc.vector.tensor_copy(q8h[:], qTm[:])
    qhb = sp.tile([P, 2, BC, NQ], BF16, name="qhb", bufs=1)
    nc.vector.tensor_copy(qhb[:], q8h[:])
    q8l = sp.tile([P, 2, BC, NQ], F8, name="q8l", bufs=1)
    nc.vector.tensor_sub(q8l[:], qTm[:], qhb[:])

    # scores: 3-term compensated fp8 DoubleRow, accumulate over b via zero cols
    att = sp.tile([NQ, S], BF16, name="att", bufs=1)
    pexp = ep.tile([NQ, NCH], F32, name="pexp")
    for j in range(NCH):
        sps = psc.tile([NQ, 512], F32, name="sc_ps", tag="seb")
        first = True
        for sub in range(2):
            reg = sps[:, sub * 256:(sub + 1) * 256]
            s0 = j * 512 + sub * 256
            for b in range(BC):
                for (qq, hl) in ((q8h, 0), (q8h, 1), (q8l, 0)):
                    nc.tensor.matmul(reg, qq[:, :, b, :],
                                     ehl[b][:, hl, :, s0:s0 + 256],
                                     start=first,
                                     stop=(sub == 1 and b == BC - 1
                                           and hl == 0 and qq is q8l),
                                     perf_mode=DR)
                    first = False
        nc.scalar.activation(att[:, j * 512:(j + 1) * 512], sps[:], Exp,
                             accum_out=pexp[:, j:j + 1])
    sm = ep.tile([NQ, 1], F32, name="sm")
    nc.vector.reduce_sum(sm[:], pexp[:], axis=mybir.AxisListType.X)
    rs = ep.tile([NQ, 1], F32, name="rs")
    nc.vector.reciprocal(rs[:], sm[:])
    nc.vector.tensor_scalar_mul(att[:], att[:], rs[:])
    # transpose attn to [s-partition] tiles
    attT = []
    for j in range(16):
        pt = psm.tile([P, NQ], BF16, name="pt_at", tag="ps")
        nc.tensor.transpose(pt[:], att[:, j * P:(j + 1) * P], id_bf[:NQ, :NQ])
        aj = sp.tile([P, NQ], BF16, name=f"attT{j}", bufs=1)
        nc.vector.tensor_copy(aj[:], pt[:])
        attT.append(aj)
    # mix: one [NQ, E] accumulation per b (bf16)
    mixTs = [ep.tile([P, 2, BC], BF16, name=f"mixT_t{a + 1}", bufs=2)
             for a in range(3)]
    for b in range(BC):
        encSb = encsp.tile([P, 16, E], BF16, name="encS")
        nc.gpsimd.dma_start(out=encSb[:], in_=io["enc_sc"].ap()[b])
        mps = psm.tile([NQ, E], F32, name="mix_ps", tag="ps")
        for j in range(16):
            nc.tensor.matmul(mps[:], attT[j][:], encSb[:, j, :],
                             start=(j == 0), stop=(j == 15))
        mfull = ep.tile([NQ, E], BF16, name="mfull", bufs=2)
        nc.scalar.copy(mfull[:], mps[:])
        for ch in range(2):
            pt = psm.tile([P, NQ], BF16, name="pt_mx", tag="ps")
            nc.tensor.transpose(pt[:], mfull[:, ch * P:(ch + 1) * P],
                                id_bf[:NQ, :NQ])
            for a in range(3):
                nc.vector.tensor_copy(mixTs[a][:, ch, b:b + 1],
                                      pt[:, a * BC + b:a * BC + b + 1])

    def attend_out(mixT, qT, tag):
        aps = psm.tile([BC, E], F32, name="ao_ps", tag="ps")
        for ch in range(2):
            nc.tensor.matmul(aps[:], mixT[:, ch, :], Wa_mT[:, ch, :],
                             start=(ch == 0), stop=False)
        for ch in range(2):
            nc.tensor.matmul(aps[:], qT[:, ch, :], Wa_qT[:, ch, :],
                             start=False, stop=False)
        nc.tensor.matmul(aps[:], ones_bf[:], b_attn[:], start=False, stop=True)
        o = ep.tile([BC, E], BF16, name=f"out_{tag}", bufs=1)
        nc.scalar.activation(o[:], aps[:], Tanh)
        oT = transpose_to(o, BF16, id_bf, f"outT_{tag}")
        return o, oT

    out2, out2T = attend_out(mixTs[1], h2T, "t2")
    out3, out3T = attend_out(mixTs[2], h3T, "t3")
    out1, out1T = attend_out(mixTs[0], h1T, "t1")

    # t1_out = out1 @ W_rel.T + b_rel -> out[:, 0:R]
    t1ps = psm.tile([BC, R], F32, name="t1_ps", tag="ps")
    for ch in range(2):
        nc.tensor.matmul(t1ps[:], out1T[:, ch, :], W_relT[:, ch, :],
                         start=(ch == 0), stop=False)
    nc.tensor.matmul(t1ps[:], ones_bf[:], b_rel[:], start=False, stop=True)
    t1sb = ep.tile([BC, R], F32, name="t1sb")
    nc.scalar.copy(t1sb[:], t1ps[:])
    dma(out=out_ap[:, 0:R], in_=t1sb[:])

    # ---- vbias variants: vb = o @ Kv_x + b_conv, transposed to [P,2,BC] ----
    def vbias(oT, Kv, tag):
        vps = psm.tile([BC, E], F32, name="vb_ps", tag="ps")
        for ch in range(2):
            nc.tensor.matmul(vps[:], oT[:, ch, :], Kv[:, ch, :],
                             start=(ch == 0), stop=False)
        nc.tensor.matmul(vps[:], ones_bf[:], b_conv[:], start=False, stop=True)
        vsb = ep.tile([BC, E], F32, name="vb_sb", bufs=2)
        nc.vector.tensor_copy(vsb[:], vps[:])
        return transpose_to(vsb, F32, id_f32, f"vbT_{tag}")

    vbA = [vbias(out2T, kv, f"a{i}") for i, kv in enumerate((Kv_i, Kv_f, Kv_l))]
    vbB = [vbias(out3T, kv, f"b{i}") for i, kv in enumerate((Kv_i, Kv_f, Kv_l))]
    vbs_both = (vbA, vbB)

    # ---- conv (3-term compensated fp8 DoubleRow) + relu + ent heads ----
    # b-major so conv keeps pace with the per-b DMA arrivals.
    def conv_psum(b, j, half):
        ps = pcv.tile([P, 512], F32, name="conv_ps")
        for term in range(3):
            W = (K8, K8, dK5)[term]
            for w in (1, 0, 2):
                for sub in range(2):
                    s0 = j * 512 + sub * 256
                    lo = s0 + w - 1
                    ob = sub * 256
                    n = 256
                    if lo < 0:
                        lo = 0
                        ob += 1
                        n = 255
                    elif lo + n > S:
                        n = S - lo
                    rhs = ehl[b][:, 1 if term == 1 else 0, :, lo:lo + n]
                    # exactly ONE start=True per psum bank: start marks the
                    # whole 2KB bank pending-zero, so later writes into the
                    # other half replace-on-first-write
                    nc.tensor.matmul(ps[:, ob:ob + n], W[:, w, half, :, :], rhs,
                                     start=(term == 0 and w == 1 and sub == 0),
                                     stop=(term == 2 and w == 2 and sub == 1),
                                     perf_mode=DR)
        return ps

    for b in range(BC):
        for j in range(NCH):
            cps = []
            for half in range(2):
                ps = conv_psum(b, j, half)
                st = stp.tile([P, 512], BF16, name="cvst")
                nc.scalar.copy(st[:], ps[:])
                cps.append(st)
            ent_ps = pent.tile([4, 512], F32, name="ent_ps")
            for v in range(2):
                vbs = vbs_both[v]
                for half in range(2):
                    r = rp.tile([P, 512], BF16, name="relu")
                    nc.vector.tensor_scalar(r[:], cps[half][:],
                                            vbs[0][:, half, b:b + 1], 0.0,
                                            op0=mybir.AluOpType.add,
                                            op1=mybir.AluOpType.max)
                    if j == 0:
                        nc.vector.tensor_scalar(r[:, 0:1], cps[half][:, 0:1],
                                                vbs[1][:, half, b:b + 1], 0.0,
                                                op0=mybir.AluOpType.add,
                                                op1=mybir.AluOpType.max)
                    if j == NCH - 1:
                        nc.vector.tensor_scalar(r[:, 511:512],
                                                cps[half][:, 511:512],
                                                vbs[2][:, half, b:b + 1], 0.0,
                                                op0=mybir.AluOpType.add,
                                                op1=mybir.AluOpType.max)
                    nc.tensor.matmul(ent_ps[:], WentP[:, v, half, :],
                                     r[:],
                                     start=(v == 0 and half == 0),
                                     stop=(v == 1 and half == 1))
            esb = ep.tile([4, 512], F32, name="esb", bufs=3)
            nc.scalar.activation(esb[:], ent_ps[:], Ident, bias=bent4[:])
            dma(out=outv[b, :, j * 512:(j + 1) * 512], in_=esb[:])


def build_nc():
    nc = bacc.Bacc("TRN2", target_bir_lowering=False, debug=False)
    io = {}

    def din(name, shape, dt):
        io[name] = nc.dram_tensor(name, shape, dt, kind="ExternalInput")

    din("ehl", [BC, 128, 2, 2, S], F8)
    din("enc_sc", [BC, 128, 16, E], BF16)
    din("wblob", [128, WTOT], BF16)
    din("wblob8", [128, W8TOT], F8)
    din("wblob85", [128, W8TOT], mybir.dt.float8e5)
    din("bent4", [4, 1], F32)
    din("c0", [BC, E], F32)
    io["out"] = nc.dram_tensor("out", [BC, R + 4 * S], F32, kind="ExternalOutput")

    with ExitStack() as ctx:
        t = ctx.enter_context(tile.TileContext(nc))
        _emit(ctx, t, nc, io)
    nc.compile()
    return nc


def _pack2(w):  # [256, N] fp32 -> [128, 2, N]
    return np.ascontiguousarray(w.reshape(2, 128, -1).transpose(1, 0, 2))


def prepare_in_maps(inputs):
    bf = ml_dtypes.bfloat16
    f8 = ml_dtypes.float8_e4m3
    enc = np.asarray(inputs["encoder_o"], np.float32)

    def q8(x):
        return x.astype(f8)

    # fp8 enc splits, [B, 128, hl, t, S] with [b,p,hl,t,s] = val(enc[b,s,t*128+p])
    encT = enc.transpose(0, 2, 1)                  # [B, E, S]
    ehi = q8(encT)
    elo = q8(encT - ehi.astype(np.float32))
    # [B, E, S] -> [B, 128, 2, S]
    def esplit(a):
        return np.ascontiguousarray(a.reshape(B, 2, 128, S).transpose(0, 2, 1, 3))
    ehl = np.ascontiguousarray(
        np.stack([esplit(ehi), esplit(elo)], axis=2))   # [B,128,2,2,S]
    enc_sc = np.ascontiguousarray(
        enc.astype(bf).reshape(B, 16, 128, E).transpose(0, 2, 1, 3))

    W_ih = np.asarray(inputs["W_ih"], np.float32)
    W_hh = np.asarray(inputs["W_hh"], np.float32)
    W_attn = np.asarray(inputs["W_attn"], np.float32)
    kern = np.asarray(inputs["W_conv"], np.float32).transpose(2, 1, 0)  # [3,2E,E]
    Kenc_ = kern[:, :E, :]
    Kv = kern[:, E:, :]
    Kv_i, Kv_f, Kv_l = Kv.sum(0), Kv[1] + Kv[2], Kv[0] + Kv[1]

    # conv fp8 weights, DoubleRow layout [128, w, co_half, ci_t, 128m]
    f85 = ml_dtypes.float8_e5m2
    K8 = q8(Kenc_)
    dK5 = (Kenc_ - K8.astype(np.float32)).astype(f85)
    def kpack(K):   # [3, 2E, E] fp8 -> [128, 3, 2, 2, 128]
        return np.ascontiguousarray(
            K.reshape(3, 2, 128, 2, 128).transpose(2, 0, 3, 1, 4))
    K8p, dK5p = kpack(K8), kpack(dK5)

    We = np.stack([np.asarray(inputs["W_ent1"])[0],
                   np.asarray(inputs["W_ent2"])[0]], 1)   # [E, 2]
    # WentP [128, v2, half2, 4]: [p, v, h, c] = We[h*128+p, c-2v] for c in {2v,2v+1}
    WentP = np.zeros((128, 2, 2, 4), np.float32)
    for v in range(2):
        for h in range(2):
            for head in range(2):
                WentP[:, v, h, 2 * v + head] = We[h * 128:(h + 1) * 128, head]

    x1 = np.broadcast_to(np.asarray(inputs["sos_emb"])[0], (B, E))
    x2 = np.asarray(inputs["rel_emb"])[np.asarray(inputs["r_in"]).astype(np.int64)]
    idx = np.arange(B)
    k1 = np.asarray(inputs["k1"])[:, 0].astype(np.int64)
    k2 = np.asarray(inputs["k2"])[:, 0].astype(np.int64)
    x3 = enc[idx, k1] + enc[idx, k2]
    X = np.stack([x1, x2, x3], 0).astype(np.float32)      # [3,B,E]
    h0 = np.asarray(inputs["h0"], np.float32)[0]
    c0 = np.asarray(inputs["c0"], np.float32)

    wsh = np.zeros((128, WTOT), np.float32)

    def put(name, arr):
        o, n = WOFF[name]
        wsh[:, o:o + n] = arr.reshape(128, n)

    def putrow(name, vec):
        o, n = WOFF[name]
        wsh[0, o:o + n] = vec.ravel()

    put("W_ihT", _pack2(W_ih.T))
    put("W_hhT", _pack2(W_hh.T))
    put("Wa_mT", _pack2(W_attn[:, :E].T))
    put("Wa_qT", _pack2(W_attn[:, E:].T))
    put("Kv_i", _pack2(Kv_i))
    put("Kv_f", _pack2(Kv_f))
    put("Kv_l", _pack2(Kv_l))
    put("W_relT", _pack2(np.asarray(inputs["W_rel"], np.float32).T))
    put("WentP", WentP)
    putrow("bias_g", np.asarray(inputs["b_ih"], np.float32)
           + np.asarray(inputs["b_hh"], np.float32))
    putrow("b_attn", np.asarray(inputs["b_attn"], np.float32))
    putrow("b_conv", np.asarray(inputs["b_conv"], np.float32))
    putrow("b_rel", np.asarray(inputs["b_rel"], np.float32))
    b1 = np.asarray(inputs["b_ent1"]).ravel()[0]
    b2 = np.asarray(inputs["b_ent2"]).ravel()[0]
    bent4 = np.array([[b1], [b2], [b1], [b2]], np.float32)

    w8 = K8p.reshape(128, W8TOT)
    w85 = dK5p.reshape(128, W8TOT)

    in_maps = []
    for c in range(NCORES):
        sl = slice(c * BC, (c + 1) * BC)
        w = wsh.copy()
        xs = X[:, sl]                                      # [3,BC,E]
        xo, xn = WOFF["xT"]
        w[:, xo:xo + xn] = xs.transpose(2, 0, 1).reshape(
            2, 128, 3, BC).transpose(1, 2, 0, 3).reshape(128, xn)
        ho, hn = WOFF["h0T"]
        w[:, ho:ho + hn] = h0[sl].T.reshape(2, 128, BC).transpose(
            1, 0, 2).reshape(128, hn)
        m = {
            "ehl": np.ascontiguousarray(ehl[sl]),
            "enc_sc": np.ascontiguousarray(enc_sc[sl]),
            "wblob": w.astype(bf),
            "wblob8": w8.astype(f8),
            "wblob85": w85.astype(f85),
            "bent4": bent4,
            "c0": np.ascontiguousarray(c0[0, sl]) if c0.ndim == 3
            else np.ascontiguousarray(c0[sl]),
        }
        in_maps.append(m)
    return in_maps


_NC_CACHE = {}


def get_nc():
    if "nc" not in _NC_CACHE:
        _NC_CACHE["nc"] = build_nc()
    return _NC_CACHE["nc"]


def kernel(**inputs) -> np.ndarray:
    nc = get_nc()
    in_maps = prepare_in_maps(inputs)
    res = run_bass_kernel_spmd(nc, in_maps, core_ids=list(range(NCORES)))
    return np.concatenate([r["out"] for r in res.results], 0).astype(np.float32)


if __name__ == "__main__":
    import jax
    import reference as refmod
    with jax.default_device(jax.devices("cpu")[0]):
        inputs = {k: np.asarray(v) for k, v in refmod.setup_inputs().items()}
        expected = np.asarray(refmod.reference(**inputs))
    actual = kernel(**inputs)
    err = np.abs(actual - expected)
    print("max abs err:", err.max(), "rel:", err.max() / np.abs(expected).max())


# revision 21
# speedup vs baseline: 1.2341x; 1.0542x over previous
"""Trainium2 Bass kernel for nn_Decoder (3-step LSTM decoder w/ Luong attention
+ conv1d entity heads). Data-parallel over batch: B=64 -> 8 cores x 8.

Decomposition (validated vs reference):
  - conv1d over feat=[enc, broadcast(o)] splits into a 3-tap matmul conv over
    enc (shared by both ent_heads calls) plus a per-batch bias vec@Kvec (with
    first/last-column variants for the SAME-padding edges).
  - attend(q) = tanh(mix @ Wa[:, :E].T + q @ Wa[:, E:].T + b) with
    mix = softmax(q.enc) @ enc.
Precision strategy (numerically validated in numsim.py, ~5e-3 rel err):
  - conv over enc in fp8 DoubleRow with 3-term compensation:
      conv = K8.ehi + K8.elo + dK16.(enc/16),  ehi=fp8(enc), elo=fp8(enc-ehi)
  - scores in fp8 DoubleRow, 3-term: qh.ehi + qh.elo + ql.ehi
  - mix / gates / attend / vbias / ent-head matmuls in bf16 (fp32 PSUM).
Entity-head outputs for (b,j) pack 4 rows (e1a,e2a,e1b,e2b) in one PSUM tile
via zero-padded Went weights -> one Act copy + one DMA per (b,j).
"""
import numpy as np
import ml_dtypes
from contextlib import ExitStack

import concourse.bass as bass
import concourse.bacc as bacc
import concourse.tile as tile
from concourse import mybir
from concourse.bass_utils import run_bass_kernel_spmd
from concourse.masks import make_identity

B, S, E, R = 64, 2048, 256, 50
NCORES = 8
BC = B // NCORES          # batch per core = 8
NCH = S // 512            # 4 s-chunks of 512
NQ = 3 * BC               # 24 attention queries
F32 = mybir.dt.float32
BF16 = mybir.dt.bfloat16
F8 = mybir.dt.float8e4
DR = mybir.MatmulPerfMode.DoubleRow
Relu = mybir.ActivationFunctionType.Relu
Tanh = mybir.ActivationFunctionType.Tanh
Exp = mybir.ActivationFunctionType.Exp
Ident = mybir.ActivationFunctionType.Identity

# packed bf16 weight blob layout: name -> (col offset, n cols) in [128, WTOT].
# part 1 (cols < WCRIT) is the LSTM-critical prefix, DMA'd first.
_WLAYOUT = [("xT", 48), ("h0T", 16), ("bias_g", 1024),
            ("W_ihT", 2048), ("W_hhT", 2048),
            ("Wa_mT", 512), ("Wa_qT", 512),
            ("Kv_i", 512), ("Kv_f", 512), ("Kv_l", 512),
            ("W_relT", 2 * R), ("WentP", 16),
            ("b_attn", 256), ("b_conv", 256), ("b_rel", R)]
WCRIT0 = 48 + 16 + 1024                  # 1088: gates-t1 critical
WCRIT = WCRIT0 + 2048 + 2048             # 5184
WOFF = {}
_o = 0
for _n, _c in _WLAYOUT:
    WOFF[_n] = (_o, _c)
    _o += _c
WTOT = _o

# fp8 conv weights in DoubleRow layout: K8 (e4m3) + dK (e5m2, the
# K-quantization residual; small values sit in e5m2's normal range)
W8TOT = 1536


def _emit(ctx, tc, nc, io):
    P = 128
    wp = ctx.enter_context(tc.tile_pool(name="wp", bufs=1))
    ep = ctx.enter_context(tc.tile_pool(name="ep", bufs=1))
    sp = ctx.enter_context(tc.tile_pool(name="sp", bufs=2))
    bigp = ctx.enter_context(tc.tile_pool(name="bigp", bufs=1))
    encsp = ctx.enter_context(tc.tile_pool(name="encsp", bufs=4))
    stp = ctx.enter_context(tc.tile_pool(name="stp", bufs=46))
    rp = ctx.enter_context(tc.tile_pool(name="rp", bufs=6))
    psc = ctx.enter_context(tc.tile_pool(name="psc", bufs=2, space="PSUM"))
    pcv = ctx.enter_context(tc.tile_pool(name="pcv", bufs=3, space="PSUM"))
    psm = ctx.enter_context(tc.tile_pool(name="psm", bufs=3, space="PSUM"))

    dma = nc.sync.dma_start

    # ---- weights / constants ----
    wsb = wp.tile([P, WTOT], BF16, name="wblob")
    w8sb = wp.tile([P, W8TOT], F8, name="wblob8")
    w85sb = wp.tile([P, W8TOT], mybir.dt.float8e5, name="wblob85")
    bent4 = wp.tile([4, 1], F32, name="bent4")
    c0 = wp.tile([BC, E], F32, name="c0")
    dma(out=wsb[:, 0:WCRIT0], in_=io["wblob"].ap()[:, 0:WCRIT0])
    dma(out=c0[:], in_=io["c0"].ap())
    dma(out=w8sb[:], in_=io["wblob8"].ap())
    dma(out=w85sb[:], in_=io["wblob85"].ap())

    # ---- enc fp8 (ehi, elo) resident tiles; e16/encS pooled with their DMAs
    # emitted next to their consumers (separate queues) so pool reuse sees
    # the readers.
    ehl = []
    for b in range(BC):
        t = bigp.tile([P, 2, 2, S], F8, name=f"ehl{b}")
        dma(out=t[:], in_=io["ehl"].ap()[b])
        ehl.append(t)
        if b == 0:
            dma(out=wsb[:, WCRIT0:WCRIT], in_=io["wblob"].ap()[:, WCRIT0:WCRIT])
            dma(out=bent4[:], in_=io["bent4"].ap())
        if b == 2:
            dma(out=wsb[:, WCRIT:], in_=io["wblob"].ap()[:, WCRIT:])

    def wview(name, *dims):
        o, n = WOFF[name]
        v = wsb[:, o:o + n]
        if not dims:
            return v
        pat = "p (" + " ".join(f"d{i}" for i in range(len(dims) + 1)) + ") -> p " \
            + " ".join(f"d{i}" for i in range(len(dims) + 1))
        return v.rearrange(pat, **{f"d{i}": d for i, d in enumerate(dims)})

    def w8view(sb, *dims):
        pat = "p (" + " ".join(f"d{i}" for i in range(len(dims) + 1)) + ") -> p " \
            + " ".join(f"d{i}" for i in range(len(dims) + 1))
        return sb[:, :].rearrange(pat, **{f"d{i}": d for i, d in enumerate(dims)})

    def brow(name):
        o, n = WOFF[name]
        return wsb[0:1, o:o + n]

    W_ihT = wview("W_ihT", 2)
    W_hhT = wview("W_hhT", 2)
    Wa_mT = wview("Wa_mT", 2)
    Wa_qT = wview("Wa_qT", 2)
    Kv_i = wview("Kv_i", 2)
    Kv_f = wview("Kv_f", 2)
    Kv_l = wview("Kv_l", 2)
    W_relT = wview("W_relT", 2)
    WentP = wview("WentP", 2, 2)       # [128, v2, half2, 4]
    xT = wview("xT", 3, 2)
    h0T = wview("h0T", 2)
    bias_g = brow("bias_g")
    b_attn = brow("b_attn")
    b_conv = brow("b_conv")
    b_rel = brow("b_rel")
    K8 = w8view(w8sb, 3, 2, 2)         # [128, w, co_half, ci_t, 128]
    dK5 = w8view(w85sb, 3, 2, 2)

    # conv: 3-term compensated fp8 DoubleRow into one [128,512] psum per
    # (b, j, co-half). Exactly ONE start=True per psum bank: start marks the
    # whole 2KB bank pending-zero, so later writes into the other half
    # replace-on-first-write.
    stages = {}

    def conv_psum(b, j, half):
        ps = pcv.tile([P, 512], F32, name="conv_ps")
        for term in range(3):
            W = (K8, K8, dK5)[term]
            for w in (1, 0, 2):
                for sub in range(2):
                    s0 = j * 512 + sub * 256
                    lo = s0 + w - 1
                    ob = sub * 256
                    n = 256
                    if lo < 0:
                        lo = 0
                        ob += 1
                        n = 255
                    elif lo + n > S:
                        n = S - lo
                    rhs = ehl[b][:, 1 if term == 1 else 0, :, lo:lo + n]
                    nc.tensor.matmul(ps[:, ob:ob + n], W[:, w, half, :, :], rhs,
                                     start=(term == 0 and w == 1 and sub == 0),
                                     stop=(term == 2 and w == 2 and sub == 1),
                                     perf_mode=DR)
        return ps

    _chunks = iter([(b, j) for b in range(BC) for j in range(NCH)])

    def emit_chunk(n=1):
        for _ in range(n):
            bj = next(_chunks, None)
            if bj is None:
                return
            b, j = bj
            for half in range(2):
                ps = conv_psum(b, j, half)
                st = stp.tile([P, 512], BF16, name="cvst")
                nc.scalar.copy(st[:], ps[:])
                stages[(b, j, half)] = st

    ones_bf = wp.tile([1, BC], BF16, name="ones_bf")
    nc.vector.memset(ones_bf[:], 1.0)
    id_bf = wp.tile([P, P], BF16, name="id_bf")
    make_identity(nc, id_bf[:])
    id_f32 = wp.tile([P, P], F32, name="id_f32")
    make_identity(nc, id_f32[:])

    out_ap = io["out"].ap()
    outv = out_ap[:, R:].rearrange("b (vh s) -> b vh s", vh=4)

    # ---- helper: transpose [BC, 2*P] sbuf -> [P, 2, BC] sbuf ----
    def transpose_to(src, dt, idt, name):
        dst = ep.tile([P, 2, BC], dt, name=name, bufs=2)
        for ch in range(2):
            pt = psm.tile([P, BC], dt, name="pt_tr", tag="ps")
            nc.tensor.transpose(pt[:], src[:, ch * P:(ch + 1) * P], idt[:BC, :BC])
            nc.scalar.copy(dst[:, ch, :], pt[:])
        return dst

    # ---- LSTM steps (batched over BC on partitions) ----
    def gates_half(t, hT, nch):
        gh = psm.tile([BC, 512], F32, name="gates", tag="ps")
        nc.tensor.matmul(gh[:], ones_bf[:], bias_g[:, nch * 512:(nch + 1) * 512],
                         start=True, stop=False)
        for kh in range(2):
            nc.tensor.matmul(gh[:], xT[:, t, kh, :],
                             W_ihT[:, kh, nch * 512:(nch + 1) * 512],
                             start=False, stop=False)
            nc.tensor.matmul(gh[:], hT[:, kh, :],
                             W_hhT[:, kh, nch * 512:(nch + 1) * 512],
                             start=False, stop=(kh == 1))
        return gh

    def lstm_step(t, hT, c_prev):
        g0 = gates_half(t, hT, 0)
        s_if = ep.tile([BC, 512], F32, name="s_if", bufs=1)
        nc.scalar.activation(s_if[:], g0[:], Tanh, scale=0.5)
        nc.vector.tensor_scalar(s_if[:], s_if[:], 0.5, 0.5,
                                op0=mybir.AluOpType.mult, op1=mybir.AluOpType.add)
        g1 = gates_half(t, hT, 1)
        t_g = ep.tile([BC, E], F32, name="t_g", bufs=1)
        nc.scalar.activation(t_g[:], g1[:, 0:256], Tanh)
        s_o = ep.tile([BC, E], F32, name="s_o", bufs=1)
        nc.scalar.activation(s_o[:], g1[:, 256:512], Tanh, scale=0.5)
        nc.vector.tensor_scalar(s_o[:], s_o[:], 0.5, 0.5,
                                op0=mybir.AluOpType.mult, op1=mybir.AluOpType.add)
        c2 = ep.tile([BC, E], F32, name="c2", bufs=2)
        nc.vector.tensor_mul(c2[:], s_if[:, 256:512], c_prev[:])
        tmp = ep.tile([BC, E], F32, name="tmp_ig", bufs=1)
        nc.vector.tensor_mul(tmp[:], s_if[:, 0:256], t_g[:])
        nc.vector.tensor_add(c2[:], c2[:], tmp[:])
        tc2 = ep.tile([BC, E], F32, name="tc2", bufs=1)
        nc.scalar.activation(tc2[:], c2[:], Tanh)
        h2 = ep.tile([BC, E], BF16, name="h2", bufs=2)
        nc.vector.tensor_mul(h2[:], s_o[:], tc2[:])
        h2T = transpose_to(h2, BF16, id_bf, f"h2T_{t}")
        return h2, h2T, c2

    # ---- attention: all 3 attends (q = h1, h2, h3) batched.
    # qTm columns zero except (a*BC+b) for batch b -> scores accumulate over b.
    qTm = sp.tile([P, 2, BC, NQ], BF16, name="qTm", bufs=1)
    nc.vector.memset(qTm[:], 0.0)

    def fill_qTm(a, hT):
        for ch in range(2):
            for b in range(BC):
                nc.vector.tensor_copy(qTm[:, ch, b, a * BC + b:a * BC + b + 1],
                                      hT[:, ch, b:b + 1])

    emit_chunk()                       # (0,0): first PE work, pre-LSTM
    h1, h1T, c1 = lstm_step(0, h0T, c0)
    fill_qTm(0, h1T)
    emit_chunk()                       # (0,1)
    h2, h2T, c2 = lstm_step(1, h1T, c1)
    fill_qTm(1, h2T)
    emit_chunk()                       # (0,2)
    h3, h3T, c3 = lstm_step(2, h2T, c2)
    fill_qTm(2, h3T)

    # fp8 hi/lo split of the queries for compensated fp8 scores
    q8h = sp.tile([P, 2, BC, NQ], F8, name="q8h", bufs=1)
    nc.vector.tensor_copy(q8h[:], qTm[:])
    qhb = sp.tile([P, 2, BC, NQ], BF16, name="qhb", bufs=1)
    nc.vector.tensor_copy(qhb[:], q8h[:])
    q8l = sp.tile([P, 2, BC, NQ], F8, name="q8l", bufs=1)
    nc.vector.tensor_sub(q8l[:], qTm[:], qhb[:])
    emit_chunk()                       # (0,3)

    # scores: 3-term compensated fp8 DoubleRow, accumulate over b via zero
    # cols. Two j-groups at a time (2 psum banks), b-paced with conv chunks
    # filling the DMA-arrival gaps.
    att = sp.tile([NQ, S], BF16, name="att", bufs=1)
    pexp = ep.tile([NQ, NCH], F32, name="pexp")

    def scores_pair(jlist, fills):
        tiles = {j: psc.tile([NQ, 512], F32, name="sc_ps", tag="seb")
                 for j in jlist}
        for b in range(BC):
            for j in jlist:
                first = (b == 0)
                for sub in range(2):
                    reg = tiles[j][:, sub * 256:(sub + 1) * 256]
                    s0 = j * 512 + sub * 256
                    for (qq, hl) in ((q8h, 0), (q8h, 1), (q8l, 0)):
                        nc.tensor.matmul(reg, qq[:, :, b, :],
                                         ehl[b][:, hl, :, s0:s0 + 256],
                                         start=first,
                                         stop=(sub == 1 and b == BC - 1
                                               and hl == 0 and qq is q8l),
                                         perf_mode=DR)
                        first = False
            if b in fills:
                emit_chunk()
        for j in jlist:
            nc.scalar.activation(att[:, j * 512:(j + 1) * 512], tiles[j][:],
                                 Exp, accum_out=pexp[:, j:j + 1])

    scores_pair((0, 1), fills=set(range(BC)))      # + (1,0)..(2,3)
    scores_pair((2, 3), fills=set())
    sm = ep.tile([NQ, 1], F32, name="sm")
    nc.vector.reduce_sum(sm[:], pexp[:], axis=mybir.AxisListType.X)
    rs = ep.tile([NQ, 1], F32, name="rs")
    nc.vector.reciprocal(rs[:], sm[:])
    nc.vector.tensor_scalar_mul(att[:], att[:], rs[:])
    # transpose attn to [s-partition] tiles
    attT = []
    for j in range(16):
        pt = psm.tile([P, NQ], BF16, name="pt_at", tag="ps")
        nc.tensor.transpose(pt[:], att[:, j * P:(j + 1) * P], id_bf[:NQ, :NQ])
        aj = sp.tile([P, NQ], BF16, name=f"attT{j}", bufs=1)
        nc.vector.tensor_copy(aj[:], pt[:])
        attT.append(aj)
    emit_chunk(2)                      # (3,2),(3,3)

    # mix: one [NQ, E] accumulation per b (bf16); conv chunks interleaved
    # at the encS-DMA pacing boundaries
    mixTs = [ep.tile([P, 2, BC], BF16, name=f"mixT_t{a + 1}", bufs=2)
             for a in range(3)]

    def mix_b(b):
        encSb = encsp.tile([P, 16, E], BF16, name="encS")
        nc.sync.dma_start(out=encSb[:], in_=io["enc_sc"].ap()[b])
        mps = psm.tile([NQ, E], F32, name="mix_ps", tag="ps")
        for j in range(16):
            nc.tensor.matmul(mps[:], attT[j][:], encSb[:, j, :],
                             start=(j == 0), stop=(j == 15))
        mfull = ep.tile([NQ, E], BF16, name="mfull", bufs=2)
        nc.scalar.copy(mfull[:], mps[:])
        for ch in range(2):
            pt = psm.tile([P, NQ], BF16, name="pt_mx", tag="ps")
            nc.tensor.transpose(pt[:], mfull[:, ch * P:(ch + 1) * P],
                                id_bf[:NQ, :NQ])
            for a in range(3):
                nc.vector.tensor_copy(mixTs[a][:, ch, b:b + 1],
                                      pt[:, a * BC + b:a * BC + b + 1])

    for b in range(BC):
        mix_b(b)
        if b in (0, 2, 4, 6):
            emit_chunk()

    def attend_out(mixT, qT, tag):
        aps = psm.tile([BC, E], F32, name="ao_ps", tag="ps")
        for ch in range(2):
            nc.tensor.matmul(aps[:], mixT[:, ch, :], Wa_mT[:, ch, :],
                             start=(ch == 0), stop=False)
        for ch in range(2):
            nc.tensor.matmul(aps[:], qT[:, ch, :], Wa_qT[:, ch, :],
                             start=False, stop=False)
        nc.tensor.matmul(aps[:], ones_bf[:], b_attn[:], start=False, stop=True)
        o = ep.tile([BC, E], BF16, name=f"out_{tag}", bufs=1)
        nc.scalar.activation(o[:], aps[:], Tanh)
        oT = transpose_to(o, BF16, id_bf, f"outT_{tag}")
        return o, oT

    out2, out2T = attend_out(mixTs[1], h2T, "t2")
    emit_chunk()
    out3, out3T = attend_out(mixTs[2], h3T, "t3")
    emit_chunk()

    # ---- vbias variants: vb = o @ Kv_x + b_conv, transposed to [P,2,BC] ----
    def vbias(oT, Kv, tag):
        vps = psm.tile([BC, E], F32, name="vb_ps", tag="ps")
        for ch in range(2):
            nc.tensor.matmul(vps[:], oT[:, ch, :], Kv[:, ch, :],
                             start=(ch == 0), stop=False)
        nc.tensor.matmul(vps[:], ones_bf[:], b_conv[:], start=False, stop=True)
        vsb = ep.tile([BC, E], F32, name="vb_sb", bufs=2)
        nc.vector.tensor_copy(vsb[:], vps[:])
        return transpose_to(vsb, F32, id_f32, f"vbT_{tag}")

    vbA = [vbias(out2T, kv, f"a{i}") for i, kv in enumerate((Kv_i, Kv_f, Kv_l))]
    emit_chunk()
    vbB = [vbias(out3T, kv, f"b{i}") for i, kv in enumerate((Kv_i, Kv_f, Kv_l))]
    emit_chunk()
    out1, out1T = attend_out(mixTs[0], h1T, "t1")
    vbs_both = (vbA, vbB)

    # t1_out = out1 @ W_rel.T + b_rel -> out[:, 0:R]
    t1ps = psm.tile([BC, R], F32, name="t1_ps", tag="ps")
    for ch in range(2):
        nc.tensor.matmul(t1ps[:], out1T[:, ch, :], W_relT[:, ch, :],
                         start=(ch == 0), stop=False)
    nc.tensor.matmul(t1ps[:], ones_bf[:], b_rel[:], start=False, stop=True)
    t1sb = ep.tile([BC, R], F32, name="t1sb")
    nc.scalar.copy(t1sb[:], t1ps[:])
    dma(out=out_ap[:, 0:R], in_=t1sb[:])



    # ---- relu + ent heads (conv stages produced earlier) ----
    for b in range(BC):
        for j in range(NCH):
            emit_chunk()                # front-load (5,0)..(7,3) in k=0..11
            cps = [stages[(b, j, half)] for half in range(2)]
            ent_ps = psc.tile([4, 512], F32, name="ent_ps", tag="seb",
                              padded_shape=[128, 512])
            for v in range(2):
                vbs = vbs_both[v]
                for half in range(2):
                    r = rp.tile([P, 512], BF16, name="relu")
                    nc.vector.tensor_scalar(r[:], cps[half][:],
                                            vbs[0][:, half, b:b + 1], 0.0,
                                            op0=mybir.AluOpType.add,
                                            op1=mybir.AluOpType.max)
                    if j == 0:
                        nc.vector.tensor_scalar(r[:, 0:1], cps[half][:, 0:1],
                                                vbs[1][:, half, b:b + 1], 0.0,
                                                op0=mybir.AluOpType.add,
                                                op1=mybir.AluOpType.max)
                    if j == NCH - 1:
                        nc.vector.tensor_scalar(r[:, 511:512],
                                                cps[half][:, 511:512],
                                                vbs[2][:, half, b:b + 1], 0.0,
                                                op0=mybir.AluOpType.add,
                                                op1=mybir.AluOpType.max)
                    nc.tensor.matmul(ent_ps[:], WentP[:, v, half, :],
                                     r[:],
                                     start=(v == 0 and half == 0),
                                     stop=(v == 1 and half == 1))
            esb = ep.tile([4, 512], F32, name="esb", bufs=3)
            nc.scalar.activation(esb[:], ent_ps[:], Ident, bias=bent4[:])
            dma(out=outv[b, :, j * 512:(j + 1) * 512], in_=esb[:])


def build_nc():
    nc = bacc.Bacc("TRN2", target_bir_lowering=False, debug=False)
    io = {}

    def din(name, shape, dt):
        io[name] = nc.dram_tensor(name, shape, dt, kind="ExternalInput")

    din("ehl", [BC, 128, 2, 2, S], F8)
    din("enc_sc", [BC, 128, 16, E], BF16)
    din("wblob", [128, WTOT], BF16)
    din("wblob8", [128, W8TOT], F8)
    din("wblob85", [128, W8TOT], mybir.dt.float8e5)
    din("bent4", [4, 1], F32)
    din("c0", [BC, E], F32)
    io["out"] = nc.dram_tensor("out", [BC, R + 4 * S], F32, kind="ExternalOutput")

    with ExitStack() as ctx:
        t = ctx.enter_context(tile.TileContext(nc))
        _emit(ctx, t, nc, io)
    nc.compile()
    return nc


def _pack2(w):  # [256, N] fp32 -> [128, 2, N]
    return np.ascontiguousarray(w.reshape(2, 128, -1).transpose(1, 0, 2))


def prepare_in_maps(inputs):
    bf = ml_dtypes.bfloat16
    f8 = ml_dtypes.float8_e4m3
    enc = np.asarray(inputs["encoder_o"], np.float32)

    def q8(x):
        return x.astype(f8)

    # fp8 enc splits, [B, 128, hl, t, S] with [b,p,hl,t,s] = val(enc[b,s,t*128+p])
    encT = enc.transpose(0, 2, 1)                  # [B, E, S]
    ehi = q8(encT)
    elo = q8(encT - ehi.astype(np.float32))
    # [B, E, S] -> [B, 128, 2, S]
    def esplit(a):
        return np.ascontiguousarray(a.reshape(B, 2, 128, S).transpose(0, 2, 1, 3))
    ehl = np.ascontiguousarray(
        np.stack([esplit(ehi), esplit(elo)], axis=2))   # [B,128,2,2,S]
    enc_sc = np.ascontiguousarray(
        enc.astype(bf).reshape(B, 16, 128, E).transpose(0, 2, 1, 3))

    W_ih = np.asarray(inputs["W_ih"], np.float32)
    W_hh = np.asarray(inputs["W_hh"], np.float32)
    W_attn = np.asarray(inputs["W_attn"], np.float32)
    kern = np.asarray(inputs["W_conv"], np.float32).transpose(2, 1, 0)  # [3,2E,E]
    Kenc_ = kern[:, :E, :]
    Kv = kern[:, E:, :]
    Kv_i, Kv_f, Kv_l = Kv.sum(0), Kv[1] + Kv[2], Kv[0] + Kv[1]

    # conv fp8 weights, DoubleRow layout [128, w, co_half, ci_t, 128m]
    f85 = ml_dtypes.float8_e5m2
    K8 = q8(Kenc_)
    dK5 = (Kenc_ - K8.astype(np.float32)).astype(f85)
    def kpack(K):   # [3, 2E, E] fp8 -> [128, 3, 2, 2, 128]
        return np.ascontiguousarray(
            K.reshape(3, 2, 128, 2, 128).transpose(2, 0, 3, 1, 4))
    K8p, dK5p = kpack(K8), kpack(dK5)

    We = np.stack([np.asarray(inputs["W_ent1"])[0],
                   np.asarray(inputs["W_ent2"])[0]], 1)   # [E, 2]
    # WentP [128, v2, half2, 4]: [p, v, h, c] = We[h*128+p, c-2v] for c in {2v,2v+1}
    WentP = np.zeros((128, 2, 2, 4), np.float32)
    for v in range(2):
        for h in range(2):
            for head in range(2):
                WentP[:, v, h, 2 * v + head] = We[h * 128:(h + 1) * 128, head]

    x1 = np.broadcast_to(np.asarray(inputs["sos_emb"])[0], (B, E))
    x2 = np.asarray(inputs["rel_emb"])[np.asarray(inputs["r_in"]).astype(np.int64)]
    idx = np.arange(B)
    k1 = np.asarray(inputs["k1"])[:, 0].astype(np.int64)
    k2 = np.asarray(inputs["k2"])[:, 0].astype(np.int64)
    x3 = enc[idx, k1] + enc[idx, k2]
    X = np.stack([x1, x2, x3], 0).astype(np.float32)      # [3,B,E]
    h0 = np.asarray(inputs["h0"], np.float32)[0]
    c0 = np.asarray(inputs["c0"], np.float32)

    wsh = np.zeros((128, WTOT), np.float32)

    def put(name, arr):
        o, n = WOFF[name]
        wsh[:, o:o + n] = arr.reshape(128, n)

    def putrow(name, vec):
        o, n = WOFF[name]
        wsh[0, o:o + n] = vec.ravel()

    put("W_ihT", _pack2(W_ih.T))
    put("W_hhT", _pack2(W_hh.T))
    put("Wa_mT", _pack2(W_attn[:, :E].T))
    put("Wa_qT", _pack2(W_attn[:, E:].T))
    put("Kv_i", _pack2(Kv_i))
    put("Kv_f", _pack2(Kv_f))
    put("Kv_l", _pack2(Kv_l))
    put("W_relT", _pack2(np.asarray(inputs["W_rel"], np.float32).T))
    put("WentP", WentP)
    putrow("bias_g", np.asarray(inputs["b_ih"], np.float32)
           + np.asarray(inputs["b_hh"], np.float32))
    putrow("b_attn", np.asarray(inputs["b_attn"], np.float32))
    putrow("b_conv", np.asarray(inputs["b_conv"], np.float32))
    putrow("b_rel", np.asarray(inputs["b_rel"], np.float32))
    b1 = np.asarray(inputs["b_ent1"]).ravel()[0]
    b2 = np.asarray(inputs["b_ent2"]).ravel()[0]
    bent4 = np.array([[b1], [b2], [b1], [b2]], np.float32)

    w8 = K8p.reshape(128, W8TOT)
    w85 = dK5p.reshape(128, W8TOT)

    in_maps = []
    for c in range(NCORES):
        sl = slice(c * BC, (c + 1) * BC)
        w = wsh.copy()
        xs = X[:, sl]                                      # [3,BC,E]
        xo, xn = WOFF["xT"]
        w[:, xo:xo + xn] = xs.transpose(2, 0, 1).reshape(
            2, 128, 3, BC).transpose(1, 2, 0, 3).reshape(128, xn)
        ho, hn = WOFF["h0T"]
        w[:, ho:ho + hn] = h0[sl].T.reshape(2, 128, BC).transpose(
            1, 0, 2).reshape(128, hn)
        m = {
            "ehl": np.ascontiguousarray(ehl[sl]),
            "enc_sc": np.ascontiguousarray(enc_sc[sl]),
            "wblob": w.astype(bf),
            "wblob8": w8.astype(f8),
            "wblob85": w85.astype(f85),
            "bent4": bent4,
            "c0": np.ascontiguousarray(c0[0, sl]) if c0.ndim == 3
            else np.ascontiguousarray(c0[sl]),
        }
        in_maps.append(m)
    return in_maps


_NC_CACHE = {}


def get_nc():
    if "nc" not in _NC_CACHE:
        _NC_CACHE["nc"] = build_nc()
    return _NC_CACHE["nc"]


def kernel(**inputs) -> np.ndarray:
    nc = get_nc()
    in_maps = prepare_in_maps(inputs)
    res = run_bass_kernel_spmd(nc, in_maps, core_ids=list(range(NCORES)))
    return np.concatenate([r["out"] for r in res.results], 0).astype(np.float32)


if __name__ == "__main__":
    import jax
    import reference as refmod
    with jax.default_device(jax.devices("cpu")[0]):
        inputs = {k: np.asarray(v) for k, v in refmod.setup_inputs().items()}
        expected = np.asarray(refmod.reference(**inputs))
    actual = kernel(**inputs)
    err = np.abs(actual - expected)
    print("max abs err:", err.max(), "rel:", err.max() / np.abs(expected).max())


# revision 22
# speedup vs baseline: 1.2628x; 1.0232x over previous
"""Trainium2 Bass kernel for nn_Decoder (3-step LSTM decoder w/ Luong attention
+ conv1d entity heads). Data-parallel over batch: B=64 -> 8 cores x 8.

Decomposition (validated vs reference):
  - conv1d over feat=[enc, broadcast(o)] splits into a 3-tap matmul conv over
    enc (shared by both ent_heads calls) plus a per-batch bias vec@Kvec (with
    first/last-column variants for the SAME-padding edges).
  - attend(q) = tanh(mix @ Wa[:, :E].T + q @ Wa[:, E:].T + b) with
    mix = softmax(q.enc) @ enc.
Precision strategy (numerically validated in numsim.py, ~5e-3 rel err):
  - conv over enc in fp8 DoubleRow with 3-term compensation:
      conv = K8.ehi + K8.elo + dK16.(enc/16),  ehi=fp8(enc), elo=fp8(enc-ehi)
  - scores in fp8 DoubleRow, 3-term: qh.ehi + qh.elo + ql.ehi
  - mix / gates / attend / vbias / ent-head matmuls in bf16 (fp32 PSUM).
Entity-head outputs for (b,j) pack 4 rows (e1a,e2a,e1b,e2b) in one PSUM tile
via zero-padded Went weights -> one Act copy + one DMA per (b,j).
"""
import numpy as np
import ml_dtypes
from contextlib import ExitStack

import concourse.bass as bass
import concourse.bacc as bacc
import concourse.tile as tile
from concourse import mybir
from concourse.bass_utils import run_bass_kernel_spmd
from concourse.masks import make_identity

B, S, E, R = 64, 2048, 256, 50
NCORES = 8
BC = B // NCORES          # batch per core = 8
NCH = S // 512            # 4 s-chunks of 512
NQ = 3 * BC               # 24 attention queries
F32 = mybir.dt.float32
BF16 = mybir.dt.bfloat16
F8 = mybir.dt.float8e4
DR = mybir.MatmulPerfMode.DoubleRow
Relu = mybir.ActivationFunctionType.Relu
Tanh = mybir.ActivationFunctionType.Tanh
Exp = mybir.ActivationFunctionType.Exp
Ident = mybir.ActivationFunctionType.Identity

# packed bf16 weight blob layout: name -> (col offset, n cols) in [128, WTOT].
# part 1 (cols < WCRIT) is the LSTM-critical prefix, DMA'd first.
_WLAYOUT = [("xT", 48), ("h0T", 16), ("bias_g", 1024),
            ("W_ihT", 2048), ("W_hhT", 2048),
            ("Wa_mT", 512), ("Wa_qT", 512),
            ("Kv_i", 512), ("Kv_f", 512), ("Kv_l", 512),
            ("W_relT", 2 * R), ("WentP", 16),
            ("b_attn", 256), ("b_conv", 256), ("b_rel", R)]
WCRIT0 = 48 + 16 + 1024                  # 1088: gates-t1 critical
WCRIT = WCRIT0 + 2048 + 2048             # 5184
WOFF = {}
_o = 0
for _n, _c in _WLAYOUT:
    WOFF[_n] = (_o, _c)
    _o += _c
WTOT = _o

# fp8 conv weights in DoubleRow layout: K8 (e4m3) + dK (e5m2, the
# K-quantization residual; small values sit in e5m2's normal range)
W8TOT = 1536


def _emit(ctx, tc, nc, io):
    P = 128
    wp = ctx.enter_context(tc.tile_pool(name="wp", bufs=1))
    ep = ctx.enter_context(tc.tile_pool(name="ep", bufs=1))
    sp = ctx.enter_context(tc.tile_pool(name="sp", bufs=2))
    bigp = ctx.enter_context(tc.tile_pool(name="bigp", bufs=1))
    encsp = ctx.enter_context(tc.tile_pool(name="encsp", bufs=4))
    stp = ctx.enter_context(tc.tile_pool(name="stp", bufs=46))
    rp = ctx.enter_context(tc.tile_pool(name="rp", bufs=8))
    psc = ctx.enter_context(tc.tile_pool(name="psc", bufs=2, space="PSUM"))
    pcv = ctx.enter_context(tc.tile_pool(name="pcv", bufs=3, space="PSUM"))
    psm = ctx.enter_context(tc.tile_pool(name="psm", bufs=3, space="PSUM"))

    dma = nc.sync.dma_start

    # ---- weights / constants ----
    wsb = wp.tile([P, WTOT], BF16, name="wblob")
    w8sb = wp.tile([P, W8TOT], F8, name="wblob8")
    w85sb = wp.tile([P, W8TOT], mybir.dt.float8e5, name="wblob85")
    bent4 = wp.tile([4, 1], F32, name="bent4")
    c0 = wp.tile([BC, E], F32, name="c0")
    dma(out=wsb[:, 0:WCRIT0], in_=io["wblob"].ap()[:, 0:WCRIT0])
    dma(out=c0[:], in_=io["c0"].ap())
    dma(out=w8sb[:], in_=io["wblob8"].ap())
    dma(out=w85sb[:], in_=io["wblob85"].ap())

    # ---- enc fp8 (ehi, elo) resident tiles; e16/encS pooled with their DMAs
    # emitted next to their consumers (separate queues) so pool reuse sees
    # the readers.
    ehl = []
    for b in range(BC):
        t = bigp.tile([P, 2, 2, S], F8, name=f"ehl{b}")
        dma(out=t[:], in_=io["ehl"].ap()[b])
        ehl.append(t)
        if b == 0:
            dma(out=wsb[:, WCRIT0:WCRIT], in_=io["wblob"].ap()[:, WCRIT0:WCRIT])
            dma(out=bent4[:], in_=io["bent4"].ap())
        if b == 2:
            dma(out=wsb[:, WCRIT:], in_=io["wblob"].ap()[:, WCRIT:])

    def wview(name, *dims):
        o, n = WOFF[name]
        v = wsb[:, o:o + n]
        if not dims:
            return v
        pat = "p (" + " ".join(f"d{i}" for i in range(len(dims) + 1)) + ") -> p " \
            + " ".join(f"d{i}" for i in range(len(dims) + 1))
        return v.rearrange(pat, **{f"d{i}": d for i, d in enumerate(dims)})

    def w8view(sb, *dims):
        pat = "p (" + " ".join(f"d{i}" for i in range(len(dims) + 1)) + ") -> p " \
            + " ".join(f"d{i}" for i in range(len(dims) + 1))
        return sb[:, :].rearrange(pat, **{f"d{i}": d for i, d in enumerate(dims)})

    def brow(name):
        o, n = WOFF[name]
        return wsb[0:1, o:o + n]

    W_ihT = wview("W_ihT", 2)
    W_hhT = wview("W_hhT", 2)
    Wa_mT = wview("Wa_mT", 2)
    Wa_qT = wview("Wa_qT", 2)
    Kv_i = wview("Kv_i", 2)
    Kv_f = wview("Kv_f", 2)
    Kv_l = wview("Kv_l", 2)
    W_relT = wview("W_relT", 2)
    WentP = wview("WentP", 2, 2)       # [128, v2, half2, 4]
    xT = wview("xT", 3, 2)
    h0T = wview("h0T", 2)
    bias_g = brow("bias_g")
    b_attn = brow("b_attn")
    b_conv = brow("b_conv")
    b_rel = brow("b_rel")
    K8 = w8view(w8sb, 3, 2, 2)         # [128, w, co_half, ci_t, 128]
    dK5 = w8view(w85sb, 3, 2, 2)

    # conv: 3-term compensated fp8 DoubleRow into one [128,512] psum per
    # (b, j, co-half). Exactly ONE start=True per psum bank: start marks the
    # whole 2KB bank pending-zero, so later writes into the other half
    # replace-on-first-write.
    stages = {}

    def conv_psum(b, j, half):
        ps = pcv.tile([P, 512], F32, name="conv_ps")
        for term in range(3):
            W = (K8, K8, dK5)[term]
            for w in (1, 0, 2):
                for sub in range(2):
                    s0 = j * 512 + sub * 256
                    lo = s0 + w - 1
                    ob = sub * 256
                    n = 256
                    if lo < 0:
                        lo = 0
                        ob += 1
                        n = 255
                    elif lo + n > S:
                        n = S - lo
                    rhs = ehl[b][:, 1 if term == 1 else 0, :, lo:lo + n]
                    nc.tensor.matmul(ps[:, ob:ob + n], W[:, w, half, :, :], rhs,
                                     start=(term == 0 and w == 1 and sub == 0),
                                     stop=(term == 2 and w == 2 and sub == 1),
                                     perf_mode=DR)
        return ps

    _chunks = iter([(b, j) for b in range(BC) for j in range(NCH)])

    def emit_chunk(n=1):
        for _ in range(n):
            bj = next(_chunks, None)
            if bj is None:
                return
            b, j = bj
            for half in range(2):
                ps = conv_psum(b, j, half)
                st = stp.tile([P, 512], BF16, name="cvst")
                nc.scalar.copy(st[:], ps[:])
                stages[(b, j, half)] = st

    ones_bf = wp.tile([1, BC], BF16, name="ones_bf")
    nc.vector.memset(ones_bf[:], 1.0)
    id_bf = wp.tile([P, P], BF16, name="id_bf")
    make_identity(nc, id_bf[:])
    id_f32 = wp.tile([P, P], F32, name="id_f32")
    make_identity(nc, id_f32[:])

    out_ap = io["out"].ap()
    outv = out_ap[:, R:].rearrange("b (vh s) -> b vh s", vh=4)

    # ---- helper: transpose [BC, 2*P] sbuf -> [P, 2, BC] sbuf ----
    def transpose_to(src, dt, idt, name):
        dst = ep.tile([P, 2, BC], dt, name=name, bufs=2)
        for ch in range(2):
            pt = psm.tile([P, BC], dt, name="pt_tr", tag="ps")
            nc.tensor.transpose(pt[:], src[:, ch * P:(ch + 1) * P], idt[:BC, :BC])
            nc.scalar.copy(dst[:, ch, :], pt[:])
        return dst

    # ---- LSTM steps (batched over BC on partitions) ----
    def gates_half(t, hT, nch):
        gh = psm.tile([BC, 512], F32, name="gates", tag="ps")
        nc.tensor.matmul(gh[:], ones_bf[:], bias_g[:, nch * 512:(nch + 1) * 512],
                         start=True, stop=False)
        for kh in range(2):
            nc.tensor.matmul(gh[:], xT[:, t, kh, :],
                             W_ihT[:, kh, nch * 512:(nch + 1) * 512],
                             start=False, stop=False)
            nc.tensor.matmul(gh[:], hT[:, kh, :],
                             W_hhT[:, kh, nch * 512:(nch + 1) * 512],
                             start=False, stop=(kh == 1))
        return gh

    def lstm_step(t, hT, c_prev):
        g0 = gates_half(t, hT, 0)
        s_if = ep.tile([BC, 512], F32, name="s_if", bufs=1)
        nc.scalar.activation(s_if[:], g0[:], Tanh, scale=0.5)
        nc.vector.tensor_scalar(s_if[:], s_if[:], 0.5, 0.5,
                                op0=mybir.AluOpType.mult, op1=mybir.AluOpType.add)
        g1 = gates_half(t, hT, 1)
        t_g = ep.tile([BC, E], F32, name="t_g", bufs=1)
        nc.scalar.activation(t_g[:], g1[:, 0:256], Tanh)
        s_o = ep.tile([BC, E], F32, name="s_o", bufs=1)
        nc.scalar.activation(s_o[:], g1[:, 256:512], Tanh, scale=0.5)
        nc.vector.tensor_scalar(s_o[:], s_o[:], 0.5, 0.5,
                                op0=mybir.AluOpType.mult, op1=mybir.AluOpType.add)
        c2 = ep.tile([BC, E], F32, name="c2", bufs=2)
        nc.vector.tensor_mul(c2[:], s_if[:, 256:512], c_prev[:])
        tmp = ep.tile([BC, E], F32, name="tmp_ig", bufs=1)
        nc.vector.tensor_mul(tmp[:], s_if[:, 0:256], t_g[:])
        nc.vector.tensor_add(c2[:], c2[:], tmp[:])
        tc2 = ep.tile([BC, E], F32, name="tc2", bufs=1)
        nc.scalar.activation(tc2[:], c2[:], Tanh)
        h2 = ep.tile([BC, E], BF16, name="h2", bufs=2)
        nc.vector.tensor_mul(h2[:], s_o[:], tc2[:])
        h2T = transpose_to(h2, BF16, id_bf, f"h2T_{t}")
        return h2, h2T, c2

    # ---- attention: all 3 attends (q = h1, h2, h3) batched.
    # qTm columns zero except (a*BC+b) for batch b -> scores accumulate over b.
    qTm = sp.tile([P, 2, BC, NQ], BF16, name="qTm", bufs=1)
    nc.vector.memset(qTm[:], 0.0)

    def fill_qTm(a, hT):
        for ch in range(2):
            for b in range(BC):
                nc.vector.tensor_copy(qTm[:, ch, b, a * BC + b:a * BC + b + 1],
                                      hT[:, ch, b:b + 1])

    emit_chunk()                       # (0,0): first PE work, pre-LSTM
    h1, h1T, c1 = lstm_step(0, h0T, c0)
    fill_qTm(0, h1T)
    emit_chunk()                       # (0,1)
    h2, h2T, c2 = lstm_step(1, h1T, c1)
    fill_qTm(1, h2T)
    emit_chunk()                       # (0,2)
    h3, h3T, c3 = lstm_step(2, h2T, c2)
    fill_qTm(2, h3T)

    # fp8 hi/lo split of the queries for compensated fp8 scores
    q8h = sp.tile([P, 2, BC, NQ], F8, name="q8h", bufs=1)
    nc.vector.tensor_copy(q8h[:], qTm[:])
    qhb = sp.tile([P, 2, BC, NQ], BF16, name="qhb", bufs=1)
    nc.vector.tensor_copy(qhb[:], q8h[:])
    q8l = sp.tile([P, 2, BC, NQ], F8, name="q8l", bufs=1)
    nc.vector.tensor_sub(q8l[:], qTm[:], qhb[:])
    emit_chunk()                       # (0,3)

    # scores: 3-term compensated fp8 DoubleRow, accumulate over b via zero
    # cols. Two j-groups at a time (2 psum banks), b-paced with conv chunks
    # filling the DMA-arrival gaps.
    att = sp.tile([NQ, S], BF16, name="att", bufs=1)
    pexp = ep.tile([NQ, NCH], F32, name="pexp")

    def scores_pair(jlist, fills):
        tiles = {j: psc.tile([NQ, 512], F32, name="sc_ps", tag="seb")
                 for j in jlist}
        for b in range(BC):
            for j in jlist:
                first = (b == 0)
                for sub in range(2):
                    reg = tiles[j][:, sub * 256:(sub + 1) * 256]
                    s0 = j * 512 + sub * 256
                    for (qq, hl) in ((q8h, 0), (q8h, 1), (q8l, 0)):
                        nc.tensor.matmul(reg, qq[:, :, b, :],
                                         ehl[b][:, hl, :, s0:s0 + 256],
                                         start=first,
                                         stop=(sub == 1 and b == BC - 1
                                               and hl == 0 and qq is q8l),
                                         perf_mode=DR)
                        first = False
            if b in fills:
                emit_chunk()
        for j in jlist:
            nc.scalar.activation(att[:, j * 512:(j + 1) * 512], tiles[j][:],
                                 Exp, accum_out=pexp[:, j:j + 1])

    scores_pair((0, 1), fills=set(range(BC)))      # + (1,0)..(2,3)
    scores_pair((2, 3), fills=set())
    sm = ep.tile([NQ, 1], F32, name="sm")
    nc.vector.reduce_sum(sm[:], pexp[:], axis=mybir.AxisListType.X)
    rs = ep.tile([NQ, 1], F32, name="rs")
    nc.vector.reciprocal(rs[:], sm[:])
    nc.vector.tensor_scalar_mul(att[:], att[:], rs[:])
    # transpose attn to [s-partition] tiles
    attT = []
    for j in range(16):
        pt = psm.tile([P, NQ], BF16, name="pt_at", tag="ps")
        nc.tensor.transpose(pt[:], att[:, j * P:(j + 1) * P], id_bf[:NQ, :NQ])
        aj = sp.tile([P, NQ], BF16, name=f"attT{j}", bufs=1)
        nc.vector.tensor_copy(aj[:], pt[:])
        attT.append(aj)
    emit_chunk(2)                      # (3,2),(3,3)

    # mix: one [NQ, E] accumulation per b (bf16); conv chunks interleaved
    # at the encS-DMA pacing boundaries
    mixTs = [ep.tile([P, 2, BC], BF16, name=f"mixT_t{a + 1}", bufs=2)
             for a in range(3)]

    def mix_b(b):
        encSb = encsp.tile([P, 16, E], BF16, name="encS")
        nc.sync.dma_start(out=encSb[:], in_=io["enc_sc"].ap()[b])
        mps = psm.tile([NQ, E], F32, name="mix_ps", tag="ps")
        for j in range(16):
            nc.tensor.matmul(mps[:], attT[j][:], encSb[:, j, :],
                             start=(j == 0), stop=(j == 15))
        mfull = ep.tile([NQ, E], BF16, name="mfull", bufs=2)
        nc.scalar.copy(mfull[:], mps[:])
        for ch in range(2):
            pt = psm.tile([P, NQ], BF16, name="pt_mx", tag="ps")
            nc.tensor.transpose(pt[:], mfull[:, ch * P:(ch + 1) * P],
                                id_bf[:NQ, :NQ])
            for a in range(3):
                nc.vector.tensor_copy(mixTs[a][:, ch, b:b + 1],
                                      pt[:, a * BC + b:a * BC + b + 1])

    for b in range(BC):
        mix_b(b)
        if b in (0, 2, 4, 6):
            emit_chunk()

    def attend_out(mixT, qT, tag):
        aps = psm.tile([BC, E], F32, name="ao_ps", tag="ps")
        for ch in range(2):
            nc.tensor.matmul(aps[:], mixT[:, ch, :], Wa_mT[:, ch, :],
                             start=(ch == 0), stop=False)
        for ch in range(2):
            nc.tensor.matmul(aps[:], qT[:, ch, :], Wa_qT[:, ch, :],
                             start=False, stop=False)
        nc.tensor.matmul(aps[:], ones_bf[:], b_attn[:], start=False, stop=True)
        o = ep.tile([BC, E], BF16, name=f"out_{tag}", bufs=1)
        nc.scalar.activation(o[:], aps[:], Tanh)
        oT = transpose_to(o, BF16, id_bf, f"outT_{tag}")
        return o, oT

    out2, out2T = attend_out(mixTs[1], h2T, "t2")
    emit_chunk()
    out3, out3T = attend_out(mixTs[2], h3T, "t3")
    emit_chunk()

    # ---- vbias variants: vb = o @ Kv_x + b_conv, transposed to [P,2,BC] ----
    def vbias(oT, Kv, tag):
        vps = psm.tile([BC, E], F32, name="vb_ps", tag="ps")
        for ch in range(2):
            nc.tensor.matmul(vps[:], oT[:, ch, :], Kv[:, ch, :],
                             start=(ch == 0), stop=False)
        nc.tensor.matmul(vps[:], ones_bf[:], b_conv[:], start=False, stop=True)
        vsb = ep.tile([BC, E], F32, name="vb_sb", bufs=2)
        nc.vector.tensor_copy(vsb[:], vps[:])
        return transpose_to(vsb, F32, id_f32, f"vbT_{tag}")

    vbA = [vbias(out2T, kv, f"a{i}") for i, kv in enumerate((Kv_i, Kv_f, Kv_l))]
    emit_chunk()
    vbB = [vbias(out3T, kv, f"b{i}") for i, kv in enumerate((Kv_i, Kv_f, Kv_l))]
    emit_chunk()
    out1, out1T = attend_out(mixTs[0], h1T, "t1")
    vbs_both = (vbA, vbB)

    # t1_out = out1 @ W_rel.T + b_rel -> out[:, 0:R]
    t1ps = psm.tile([BC, R], F32, name="t1_ps", tag="ps")
    for ch in range(2):
        nc.tensor.matmul(t1ps[:], out1T[:, ch, :], W_relT[:, ch, :],
                         start=(ch == 0), stop=False)
    nc.tensor.matmul(t1ps[:], ones_bf[:], b_rel[:], start=False, stop=True)
    t1sb = ep.tile([BC, R], F32, name="t1sb")
    nc.scalar.copy(t1sb[:], t1ps[:])
    dma(out=out_ap[:, 0:R], in_=t1sb[:])



    # ---- relu + ent heads (conv stages produced earlier) ----
    k = 0
    for b in range(BC):
        for j in range(NCH):
            if k % 3 == 0:
                emit_chunk()            # spread remaining chunks evenly
            k += 1
            cps = [stages[(b, j, half)] for half in range(2)]
            ent_ps = pcv.tile([4, 512], F32, name="ent_ps", tag="conv_ps",
                              padded_shape=[128, 512])
            for v in range(2):
                vbs = vbs_both[v]
                for half in range(2):
                    r = rp.tile([P, 512], BF16, name="relu")
                    nc.vector.tensor_scalar(r[:], cps[half][:],
                                            vbs[0][:, half, b:b + 1], 0.0,
                                            op0=mybir.AluOpType.add,
                                            op1=mybir.AluOpType.max)
                    if j == 0:
                        nc.vector.tensor_scalar(r[:, 0:1], cps[half][:, 0:1],
                                                vbs[1][:, half, b:b + 1], 0.0,
                                                op0=mybir.AluOpType.add,
                                                op1=mybir.AluOpType.max)
                    if j == NCH - 1:
                        nc.vector.tensor_scalar(r[:, 511:512],
                                                cps[half][:, 511:512],
                                                vbs[2][:, half, b:b + 1], 0.0,
                                                op0=mybir.AluOpType.add,
                                                op1=mybir.AluOpType.max)
                    nc.tensor.matmul(ent_ps[:], WentP[:, v, half, :],
                                     r[:],
                                     start=(v == 0 and half == 0),
                                     stop=(v == 1 and half == 1))
            esb = ep.tile([4, 512], F32, name="esb", bufs=3)
            nc.scalar.activation(esb[:], ent_ps[:], Ident, bias=bent4[:])
            dma(out=outv[b, :, j * 512:(j + 1) * 512], in_=esb[:])


def build_nc():
    nc = bacc.Bacc("TRN2", target_bir_lowering=False, debug=False)
    io = {}

    def din(name, shape, dt):
        io[name] = nc.dram_tensor(name, shape, dt, kind="ExternalInput")

    din("ehl", [BC, 128, 2, 2, S], F8)
    din("enc_sc", [BC, 128, 16, E], BF16)
    din("wblob", [128, WTOT], BF16)
    din("wblob8", [128, W8TOT], F8)
    din("wblob85", [128, W8TOT], mybir.dt.float8e5)
    din("bent4", [4, 1], F32)
    din("c0", [BC, E], F32)
    io["out"] = nc.dram_tensor("out", [BC, R + 4 * S], F32, kind="ExternalOutput")

    with ExitStack() as ctx:
        t = ctx.enter_context(tile.TileContext(nc))
        _emit(ctx, t, nc, io)
    nc.compile()
    return nc


def _pack2(w):  # [256, N] fp32 -> [128, 2, N]
    return np.ascontiguousarray(w.reshape(2, 128, -1).transpose(1, 0, 2))


def prepare_in_maps(inputs):
    bf = ml_dtypes.bfloat16
    f8 = ml_dtypes.float8_e4m3
    enc = np.asarray(inputs["encoder_o"], np.float32)

    def q8(x):
        return x.astype(f8)

    # fp8 enc splits, [B, 128, hl, t, S] with [b,p,hl,t,s] = val(enc[b,s,t*128+p])
    encT = enc.transpose(0, 2, 1)                  # [B, E, S]
    ehi = q8(encT)
    elo = q8(encT - ehi.astype(np.float32))
    # [B, E, S] -> [B, 128, 2, S]
    def esplit(a):
        return np.ascontiguousarray(a.reshape(B, 2, 128, S).transpose(0, 2, 1, 3))
    ehl = np.ascontiguousarray(
        np.stack([esplit(ehi), esplit(elo)], axis=2))   # [B,128,2,2,S]
    enc_sc = np.ascontiguousarray(
        enc.astype(bf).reshape(B, 16, 128, E).transpose(0, 2, 1, 3))

    W_ih = np.asarray(inputs["W_ih"], np.float32)
    W_hh = np.asarray(inputs["W_hh"], np.float32)
    W_attn = np.asarray(inputs["W_attn"], np.float32)
    kern = np.asarray(inputs["W_conv"], np.float32).transpose(2, 1, 0)  # [3,2E,E]
    Kenc_ = kern[:, :E, :]
    Kv = kern[:, E:, :]
    Kv_i, Kv_f, Kv_l = Kv.sum(0), Kv[1] + Kv[2], Kv[0] + Kv[1]

    # conv fp8 weights, DoubleRow layout [128, w, co_half, ci_t, 128m]
    f85 = ml_dtypes.float8_e5m2
    K8 = q8(Kenc_)
    dK5 = (Kenc_ - K8.astype(np.float32)).astype(f85)
    def kpack(K):   # [3, 2E, E] fp8 -> [128, 3, 2, 2, 128]
        return np.ascontiguousarray(
            K.reshape(3, 2, 128, 2, 128).transpose(2, 0, 3, 1, 4))
    K8p, dK5p = kpack(K8), kpack(dK5)

    We = np.stack([np.asarray(inputs["W_ent1"])[0],
                   np.asarray(inputs["W_ent2"])[0]], 1)   # [E, 2]
    # WentP [128, v2, half2, 4]: [p, v, h, c] = We[h*128+p, c-2v] for c in {2v,2v+1}
    WentP = np.zeros((128, 2, 2, 4), np.float32)
    for v in range(2):
        for h in range(2):
            for head in range(2):
                WentP[:, v, h, 2 * v + head] = We[h * 128:(h + 1) * 128, head]

    x1 = np.broadcast_to(np.asarray(inputs["sos_emb"])[0], (B, E))
    x2 = np.asarray(inputs["rel_emb"])[np.asarray(inputs["r_in"]).astype(np.int64)]
    idx = np.arange(B)
    k1 = np.asarray(inputs["k1"])[:, 0].astype(np.int64)
    k2 = np.asarray(inputs["k2"])[:, 0].astype(np.int64)
    x3 = enc[idx, k1] + enc[idx, k2]
    X = np.stack([x1, x2, x3], 0).astype(np.float32)      # [3,B,E]
    h0 = np.asarray(inputs["h0"], np.float32)[0]
    c0 = np.asarray(inputs["c0"], np.float32)

    wsh = np.zeros((128, WTOT), np.float32)

    def put(name, arr):
        o, n = WOFF[name]
        wsh[:, o:o + n] = arr.reshape(128, n)

    def putrow(name, vec):
        o, n = WOFF[name]
        wsh[0, o:o + n] = vec.ravel()

    put("W_ihT", _pack2(W_ih.T))
    put("W_hhT", _pack2(W_hh.T))
    put("Wa_mT", _pack2(W_attn[:, :E].T))
    put("Wa_qT", _pack2(W_attn[:, E:].T))
    put("Kv_i", _pack2(Kv_i))
    put("Kv_f", _pack2(Kv_f))
    put("Kv_l", _pack2(Kv_l))
    put("W_relT", _pack2(np.asarray(inputs["W_rel"], np.float32).T))
    put("WentP", WentP)
    putrow("bias_g", np.asarray(inputs["b_ih"], np.float32)
           + np.asarray(inputs["b_hh"], np.float32))
    putrow("b_attn", np.asarray(inputs["b_attn"], np.float32))
    putrow("b_conv", np.asarray(inputs["b_conv"], np.float32))
    putrow("b_rel", np.asarray(inputs["b_rel"], np.float32))
    b1 = np.asarray(inputs["b_ent1"]).ravel()[0]
    b2 = np.asarray(inputs["b_ent2"]).ravel()[0]
    bent4 = np.array([[b1], [b2], [b1], [b2]], np.float32)

    w8 = K8p.reshape(128, W8TOT)
    w85 = dK5p.reshape(128, W8TOT)

    in_maps = []
    for c in range(NCORES):
        sl = slice(c * BC, (c + 1) * BC)
        w = wsh.copy()
        xs = X[:, sl]                                      # [3,BC,E]
        xo, xn = WOFF["xT"]
        w[:, xo:xo + xn] = xs.transpose(2, 0, 1).reshape(
            2, 128, 3, BC).transpose(1, 2, 0, 3).reshape(128, xn)
        ho, hn = WOFF["h0T"]
        w[:, ho:ho + hn] = h0[sl].T.reshape(2, 128, BC).transpose(
            1, 0, 2).reshape(128, hn)
        m = {
            "ehl": np.ascontiguousarray(ehl[sl]),
            "enc_sc": np.ascontiguousarray(enc_sc[sl]),
            "wblob": w.astype(bf),
            "wblob8": w8.astype(f8),
            "wblob85": w85.astype(f85),
            "bent4": bent4,
            "c0": np.ascontiguousarray(c0[0, sl]) if c0.ndim == 3
            else np.ascontiguousarray(c0[sl]),
        }
        in_maps.append(m)
    return in_maps


_NC_CACHE = {}


def get_nc():
    if "nc" not in _NC_CACHE:
        _NC_CACHE["nc"] = build_nc()
    return _NC_CACHE["nc"]


def kernel(**inputs) -> np.ndarray:
    nc = get_nc()
    in_maps = prepare_in_maps(inputs)
    res = run_bass_kernel_spmd(nc, in_maps, core_ids=list(range(NCORES)))
    return np.concatenate([r["out"] for r in res.results], 0).astype(np.float32)


if __name__ == "__main__":
    import jax
    import reference as refmod
    with jax.default_device(jax.devices("cpu")[0]):
        inputs = {k: np.asarray(v) for k, v in refmod.setup_inputs().items()}
        expected = np.asarray(refmod.reference(**inputs))
    actual = kernel(**inputs)
    err = np.abs(actual - expected)
    print("max abs err:", err.max(), "rel:", err.max() / np.abs(expected).max())


# revision 31
# speedup vs baseline: 1.3403x; 1.0614x over previous
"""Trainium2 Bass kernel for nn_Decoder (3-step LSTM decoder w/ Luong attention
+ conv1d entity heads). Data-parallel over batch: B=64 -> 8 cores x 8.

Decomposition (validated vs reference):
  - conv1d over feat=[enc, broadcast(o)] splits into a 3-tap matmul conv over
    enc (shared by both ent_heads calls) plus a per-batch bias vec@Kvec (with
    first/last-column variants for the SAME-padding edges).
  - attend(q) = tanh(mix @ Wa[:, :E].T + q @ Wa[:, E:].T + b) with
    mix = softmax(q.enc) @ enc.
Precision strategy (numerically validated in numsim.py, ~5e-3 rel err):
  - conv over enc in fp8 DoubleRow with 3-term compensation:
      conv = K8.ehi + K8.elo + dK16.(enc/16),  ehi=fp8(enc), elo=fp8(enc-ehi)
  - scores in fp8 DoubleRow, 3-term: qh.ehi + qh.elo + ql.ehi
  - mix / gates / attend / vbias / ent-head matmuls in bf16 (fp32 PSUM).
Entity-head outputs for (b,j) pack 4 rows (e1a,e2a,e1b,e2b) in one PSUM tile
via zero-padded Went weights -> one Act copy + one DMA per (b,j).
"""
import numpy as np
import ml_dtypes
from contextlib import ExitStack

import concourse.bass as bass
import concourse.bacc as bacc
import concourse.tile as tile
from concourse import mybir
from concourse.bass_utils import run_bass_kernel_spmd
from concourse.masks import make_identity

B, S, E, R = 64, 2048, 256, 50
NCORES = 8
BC = B // NCORES          # batch per core = 8
NCH = S // 512            # 4 s-chunks of 512
NQ = 3 * BC               # 24 attention queries
F32 = mybir.dt.float32
BF16 = mybir.dt.bfloat16
F8 = mybir.dt.float8e4
DR = mybir.MatmulPerfMode.DoubleRow
Relu = mybir.ActivationFunctionType.Relu
Tanh = mybir.ActivationFunctionType.Tanh
Exp = mybir.ActivationFunctionType.Exp
Ident = mybir.ActivationFunctionType.Identity

# packed bf16 weight blob layout: name -> (col offset, n cols) in [128, WTOT].
# part 1 (cols < WCRIT) is the LSTM-critical prefix, DMA'd first.
_WLAYOUT = [("xT", 48), ("h0T", 16), ("bias_g", 1024),
            ("W_ihT", 2048), ("W_hhT", 2048),
            ("Wa_mT", 512), ("Wa_qT", 512),
            ("Kv_i", 512), ("Kv_f", 512), ("Kv_l", 512),
            ("W_relT", 2 * R), ("WentP", 16),
            ("b_attn", 256), ("b_conv", 256), ("b_rel", R)]
WCRIT0 = 48 + 16 + 1024                  # 1088: gates-t1 critical
WCRIT = WCRIT0 + 2048 + 2048             # 5184
WOFF = {}
_o = 0
for _n, _c in _WLAYOUT:
    WOFF[_n] = (_o, _c)
    _o += _c
WTOT = _o

# fp8 conv weights in DoubleRow layout: K8 (e4m3) + dK (e5m2, the
# K-quantization residual; small values sit in e5m2's normal range)
W8TOT = 1536


def _emit(ctx, tc, nc, io):
    P = 128
    wp = ctx.enter_context(tc.tile_pool(name="wp", bufs=1))
    ep = ctx.enter_context(tc.tile_pool(name="ep", bufs=1))
    sp = ctx.enter_context(tc.tile_pool(name="sp", bufs=2))
    bigp = ctx.enter_context(tc.tile_pool(name="bigp", bufs=1))
    encsp = ctx.enter_context(tc.tile_pool(name="encsp", bufs=4))
    stp = ctx.enter_context(tc.tile_pool(name="stp", bufs=46))
    rp = ctx.enter_context(tc.tile_pool(name="rp", bufs=8))
    psc = ctx.enter_context(tc.tile_pool(name="psc", bufs=2, space="PSUM"))
    pcv = ctx.enter_context(tc.tile_pool(name="pcv", bufs=3, space="PSUM"))
    psm = ctx.enter_context(tc.tile_pool(name="psm", bufs=3, space="PSUM"))

    dma = nc.sync.dma_start

    # ---- weights / constants ----
    wsb = wp.tile([P, WTOT], BF16, name="wblob")
    w8sb = wp.tile([P, W8TOT], F8, name="wblob8")
    w85sb = wp.tile([P, W8TOT], mybir.dt.float8e5, name="wblob85")
    c0 = wp.tile([BC, E], F32, name="c0")
    dma(out=wsb[:, 0:WCRIT0], in_=io["wblob"].ap()[:, 0:WCRIT0])

    # ---- enc fp8 (ehi, elo) resident tiles; e16/encS pooled with their DMAs
    # emitted next to their consumers (separate queues) so pool reuse sees
    # the readers.
    ehl = []
    for b in range(BC):
        t = bigp.tile([P, 2, 2, S], F8, name=f"ehl{b}")
        if b == 0:
            # split b0's load so conv (0,0) can start ~4us earlier
            dma(out=t[:, :, :, 0:640], in_=io["ehl"].ap()[b][:, :, :, 0:640])
            dma(out=w8sb[:], in_=io["wblob8"].ap())
            dma(out=w85sb[:], in_=io["wblob85"].ap())
            dma(out=t[:, :, :, 640:S], in_=io["ehl"].ap()[b][:, :, :, 640:S])
            dma(out=c0[:], in_=io["c0"].ap())
            dma(out=wsb[:, WCRIT0:WCRIT], in_=io["wblob"].ap()[:, WCRIT0:WCRIT])
        else:
            dma(out=t[:], in_=io["ehl"].ap()[b])
        ehl.append(t)
        if b == 2:
            dma(out=wsb[:, WCRIT:], in_=io["wblob"].ap()[:, WCRIT:])

    def wview(name, *dims):
        o, n = WOFF[name]
        v = wsb[:, o:o + n]
        if not dims:
            return v
        pat = "p (" + " ".join(f"d{i}" for i in range(len(dims) + 1)) + ") -> p " \
            + " ".join(f"d{i}" for i in range(len(dims) + 1))
        return v.rearrange(pat, **{f"d{i}": d for i, d in enumerate(dims)})

    def w8view(sb, *dims):
        pat = "p (" + " ".join(f"d{i}" for i in range(len(dims) + 1)) + ") -> p " \
            + " ".join(f"d{i}" for i in range(len(dims) + 1))
        return sb[:, :].rearrange(pat, **{f"d{i}": d for i, d in enumerate(dims)})

    def brow(name):
        o, n = WOFF[name]
        return wsb[0:1, o:o + n]

    W_ihT = wview("W_ihT", 2)
    W_hhT = wview("W_hhT", 2)
    Wa_mT = wview("Wa_mT", 2)
    Wa_qT = wview("Wa_qT", 2)
    Kv_i = wview("Kv_i", 2)
    Kv_f = wview("Kv_f", 2)
    Kv_l = wview("Kv_l", 2)
    W_relT = wview("W_relT", 2)
    WentP = wview("WentP", 2, 2)       # [128, v2, half2, 4]
    xT = wview("xT", 3, 2)
    h0T = wview("h0T", 2)
    bias_g = brow("bias_g")
    b_attn = brow("b_attn")
    b_conv = brow("b_conv")
    b_rel = brow("b_rel")
    K8 = w8view(w8sb, 3, 2, 2)         # [128, w, co_half, ci_t, 128]
    dK5 = w8view(w85sb, 3, 2, 2)

    # conv: 3-term compensated fp8 DoubleRow into one [128,512] psum per
    # (b, j, co-half). Exactly ONE start=True per psum bank: start marks the
    # whole 2KB bank pending-zero, so later writes into the other half
    # replace-on-first-write.
    stages = {}

    def conv_psum(b, j, half):
        ps = pcv.tile([P, 512], F32, name="conv_ps")
        for term in range(3):
            W = (K8, K8, dK5)[term]
            for w in (1, 0, 2):
                for sub in range(2):
                    s0 = j * 512 + sub * 256
                    lo = s0 + w - 1
                    ob = sub * 256
                    n = 256
                    if lo < 0:
                        lo = 0
                        ob += 1
                        n = 255
                    elif lo + n > S:
                        n = S - lo
                    rhs = ehl[b][:, 1 if term == 1 else 0, :, lo:lo + n]
                    nc.tensor.matmul(ps[:, ob:ob + n], W[:, w, half, :, :], rhs,
                                     start=(term == 0 and w == 1 and sub == 0),
                                     stop=(term == 2 and w == 2 and sub == 1),
                                     perf_mode=DR)
        return ps

    _chunks = iter([(b, j) for b in range(BC) for j in range(NCH)])

    def emit_chunk(n=1):
        for _ in range(n):
            bj = next(_chunks, None)
            if bj is None:
                return
            b, j = bj
            for half in range(2):
                ps = conv_psum(b, j, half)
                st = stp.tile([P, 512], BF16, name="cvst")
                nc.scalar.copy(st[:], ps[:])
                stages[(b, j, half)] = st

    ones_bf = wp.tile([1, BC], BF16, name="ones_bf")
    nc.vector.memset(ones_bf[:], 1.0)
    id_bf = wp.tile([P, P], BF16, name="id_bf")
    make_identity(nc, id_bf[:])
    id_f32 = wp.tile([P, P], F32, name="id_f32")
    make_identity(nc, id_f32[:])
    id_f8 = wp.tile([NQ, NQ], F8, name="id_f8")
    nc.vector.tensor_copy(id_f8[:], id_bf[:NQ, :NQ])

    out_ap = io["out"].ap()
    outv = out_ap[:, R:].rearrange("b (vh s) -> b vh s", vh=4)

    # ---- helper: transpose [BC, 2*P] sbuf -> [P, 2, BC] sbuf ----
    def transpose_to(src, dt, idt, name):
        dst = ep.tile([P, 2, BC], dt, name=name, bufs=2)
        for ch in range(2):
            pt = psm.tile([P, BC], dt, name="pt_tr", tag="ps")
            nc.tensor.transpose(pt[:], src[:, ch * P:(ch + 1) * P], idt[:BC, :BC])
            nc.scalar.copy(dst[:, ch, :], pt[:])
        return dst

    # ---- LSTM steps (batched over BC on partitions) ----
    def gates_half(t, hT, nch):
        gh = psm.tile([BC, 512], F32, name="gates", tag="ps")
        nc.tensor.matmul(gh[:], ones_bf[:], bias_g[:, nch * 512:(nch + 1) * 512],
                         start=True, stop=False)
        for kh in range(2):
            nc.tensor.matmul(gh[:], xT[:, t, kh, :],
                             W_ihT[:, kh, nch * 512:(nch + 1) * 512],
                             start=False, stop=False)
            nc.tensor.matmul(gh[:], hT[:, kh, :],
                             W_hhT[:, kh, nch * 512:(nch + 1) * 512],
                             start=False, stop=(kh == 1))
        return gh

    def lstm_step(t, hT, c_prev):
        g0 = gates_half(t, hT, 0)
        s_if = ep.tile([BC, 512], F32, name="s_if", bufs=1)
        nc.scalar.activation(s_if[:], g0[:], Tanh, scale=0.5)
        nc.vector.tensor_scalar(s_if[:], s_if[:], 0.5, 0.5,
                                op0=mybir.AluOpType.mult, op1=mybir.AluOpType.add)
        g1 = gates_half(t, hT, 1)
        t_g = ep.tile([BC, E], F32, name="t_g", bufs=1)
        nc.scalar.activation(t_g[:], g1[:, 0:256], Tanh)
        s_o = ep.tile([BC, E], F32, name="s_o", bufs=1)
        nc.scalar.activation(s_o[:], g1[:, 256:512], Tanh, scale=0.5)
        nc.vector.tensor_scalar(s_o[:], s_o[:], 0.5, 0.5,
                                op0=mybir.AluOpType.mult, op1=mybir.AluOpType.add)
        c2 = ep.tile([BC, E], F32, name="c2", bufs=2)
        nc.vector.tensor_mul(c2[:], s_if[:, 256:512], c_prev[:])
        tmp = ep.tile([BC, E], F32, name="tmp_ig", bufs=1)
        nc.vector.tensor_mul(tmp[:], s_if[:, 0:256], t_g[:])
        nc.vector.tensor_add(c2[:], c2[:], tmp[:])
        tc2 = ep.tile([BC, E], F32, name="tc2", bufs=1)
        nc.scalar.activation(tc2[:], c2[:], Tanh)
        h2 = ep.tile([BC, E], BF16, name="h2", bufs=2)
        nc.vector.tensor_mul(h2[:], s_o[:], tc2[:])
        h2T = transpose_to(h2, BF16, id_bf, f"h2T_{t}")
        return h2, h2T, c2

    # ---- attention: all 3 attends (q = h1, h2, h3) batched.
    # qTm columns zero except (a*BC+b) for batch b -> scores accumulate over b.
    qTm = sp.tile([P, 2, BC, NQ], BF16, name="qTm", bufs=1)
    nc.vector.memset(qTm[:], 0.0)

    def fill_qTm(a, hT):
        for ch in range(2):
            for b in range(BC):
                nc.vector.tensor_copy(qTm[:, ch, b, a * BC + b:a * BC + b + 1],
                                      hT[:, ch, b:b + 1])

    emit_chunk()                       # (0,0): first PE work, pre-LSTM
    h1, h1T, c1 = lstm_step(0, h0T, c0)
    fill_qTm(0, h1T)
    emit_chunk(2)                      # (0,1),(0,2)
    h2, h2T, c2 = lstm_step(1, h1T, c1)
    fill_qTm(1, h2T)
    emit_chunk(2)                      # (0,3),(1,0)
    h3, h3T, c3 = lstm_step(2, h2T, c2)
    fill_qTm(2, h3T)

    # fp8 hi/lo split of the queries for compensated fp8 scores
    q8h = sp.tile([P, 2, BC, NQ], F8, name="q8h", bufs=1)
    nc.vector.tensor_copy(q8h[:], qTm[:])
    qhb = sp.tile([P, 2, BC, NQ], BF16, name="qhb", bufs=1)
    nc.vector.tensor_copy(qhb[:], q8h[:])
    q8l = sp.tile([P, 2, BC, NQ], F8, name="q8l", bufs=1)
    nc.vector.tensor_sub(q8l[:], qTm[:], qhb[:])
    emit_chunk()                       # (0,3)

    # scores: 3-term compensated fp8 DoubleRow, accumulate over b via zero
    # cols. Two j-groups at a time (2 psum banks), b-paced with conv chunks
    # filling the DMA-arrival gaps.
    att = sp.tile([NQ, S], BF16, name="att", bufs=1)
    pexp = ep.tile([NQ, NCH], F32, name="pexp")

    def scores_pair(jlist, fills):
        tiles = {j: psc.tile([NQ, 512], F32, name="sc_ps", tag="seb")
                 for j in jlist}
        for b in range(BC):
            for j in jlist:
                first = (b == 0)
                for sub in range(2):
                    reg = tiles[j][:, sub * 256:(sub + 1) * 256]
                    s0 = j * 512 + sub * 256
                    for (qq, hl) in ((q8h, 0), (q8h, 1), (q8l, 0)):
                        nc.tensor.matmul(reg, qq[:, :, b, :],
                                         ehl[b][:, hl, :, s0:s0 + 256],
                                         start=first,
                                         stop=(sub == 1 and b == BC - 1
                                               and hl == 0 and qq is q8l),
                                         perf_mode=DR)
                        first = False
            if b in fills:
                emit_chunk()
        for j in jlist:
            nc.scalar.activation(att[:, j * 512:(j + 1) * 512], tiles[j][:],
                                 Exp, accum_out=pexp[:, j:j + 1])

    scores_pair((0, 1), fills=set(range(BC)))      # + (1,0)..(2,3)
    scores_pair((2, 3), fills=set())
    sm = ep.tile([NQ, 1], F32, name="sm")
    nc.vector.reduce_sum(sm[:], pexp[:], axis=mybir.AxisListType.X)
    rs = ep.tile([NQ, 1], F32, name="rs")
    nc.vector.reciprocal(rs[:], sm[:])
    # normalized, x64-scaled fp8 hi/lo split of the attention weights
    # (mix = (ah.ehi + al.ehi + ah.elo)/64, fp8 DoubleRow)
    rs64 = ep.tile([NQ, 1], F32, name="rs64")
    nc.vector.tensor_scalar_mul(rs64[:], rs[:], 64.0)
    an64 = sp.tile([NQ, S], BF16, name="an64", bufs=1)
    # last dim padded 24->64 so DoubleRow k-tile weight stride is 64B-aligned
    attT = [sp.tile([P, 16, 64], F8, name="attTh", bufs=1),
            sp.tile([P, 16, 64], F8, name="attTl", bufs=1)]
    # bf16 transpose, fp8 hi/lo split during the tiny psum->sbuf copies
    for c in range(4):
        cs = slice(c * 512, (c + 1) * 512)
        nc.vector.tensor_scalar_mul(an64[:, cs], att[:, cs], rs64[:])
        for jj in range(4):
            j = c * 4 + jj
            pt = psm.tile([P, NQ], BF16, name="pt_at", tag="ps")
            nc.tensor.transpose(pt[:], an64[:, j * P:(j + 1) * P],
                                id_bf[:NQ, :NQ])
            nc.vector.tensor_copy(attT[0][:, j, 0:NQ], pt[:])
            nc.vector.tensor_sub(attT[1][:, j, 0:NQ], pt[:],
                                 attT[0][:, j, 0:NQ])
    emit_chunk(2)                      # (3,2),(3,3)

    # mix: one [NQ, E] accumulation per b (bf16); conv chunks interleaved
    # at the encS-DMA pacing boundaries
    mixTs = [ep.tile([P, 2, BC], BF16, name=f"mixT_t{a + 1}", bufs=2)
             for a in range(3)]

    def mix_b(b):
        encSb = encsp.tile([P, 2, 16, E], F8, name="encS")
        nc.sync.dma_start(out=encSb[:], in_=io["enc_sc"].ap()[b])
        mps = psm.tile([NQ, E], F32, name="mix_ps", tag="ps")
        first = True
        for aT, hl in ((attT[0], 0), (attT[1], 0), (attT[0], 1)):
            for k in range(8):
                nc.tensor.matmul(mps[:], aT[:, 2 * k:2 * k + 2, 0:NQ],
                                 encSb[:, hl, 2 * k:2 * k + 2, :],
                                 start=first,
                                 stop=(hl == 1 and k == 7),
                                 perf_mode=DR)
                first = False
        mfull = ep.tile([NQ, E], BF16, name="mfull", bufs=2)
        nc.scalar.mul(mfull[:], mps[:], 1.0 / 64.0)
        for ch in range(2):
            pt = psm.tile([P, NQ], BF16, name="pt_mx", tag="ps")
            nc.tensor.transpose(pt[:], mfull[:, ch * P:(ch + 1) * P],
                                id_bf[:NQ, :NQ])
            for a in range(3):
                nc.vector.tensor_copy(mixTs[a][:, ch, b:b + 1],
                                      pt[:, a * BC + b:a * BC + b + 1])

    for b in range(BC):
        mix_b(b)
        if b in (0, 2, 4, 6):
            emit_chunk()

    def attend_out(mixT, qT, tag):
        aps = psm.tile([BC, E], F32, name="ao_ps", tag="ps")
        for ch in range(2):
            nc.tensor.matmul(aps[:], mixT[:, ch, :], Wa_mT[:, ch, :],
                             start=(ch == 0), stop=False)
        for ch in range(2):
            nc.tensor.matmul(aps[:], qT[:, ch, :], Wa_qT[:, ch, :],
                             start=False, stop=False)
        nc.tensor.matmul(aps[:], ones_bf[:], b_attn[:], start=False, stop=True)
        o = ep.tile([BC, E], BF16, name=f"out_{tag}", bufs=1)
        nc.scalar.activation(o[:], aps[:], Tanh)
        oT = transpose_to(o, BF16, id_bf, f"outT_{tag}")
        return o, oT

    out2, out2T = attend_out(mixTs[1], h2T, "t2")
    emit_chunk()
    out3, out3T = attend_out(mixTs[2], h3T, "t3")
    emit_chunk()

    # ---- vbias variants: vb = o @ Kv_x + b_conv, transposed to [P,2,BC] ----
    def vbias(oT, Kv, tag):
        vps = psm.tile([BC, E], F32, name="vb_ps", tag="ps")
        for ch in range(2):
            nc.tensor.matmul(vps[:], oT[:, ch, :], Kv[:, ch, :],
                             start=(ch == 0), stop=False)
        nc.tensor.matmul(vps[:], ones_bf[:], b_conv[:], start=False, stop=True)
        vsb = ep.tile([BC, E], F32, name="vb_sb", bufs=2)
        nc.vector.tensor_copy(vsb[:], vps[:])
        return transpose_to(vsb, F32, id_f32, f"vbT_{tag}")

    vbA = [vbias(out2T, kv, f"a{i}") for i, kv in enumerate((Kv_i, Kv_f, Kv_l))]
    emit_chunk()
    vbB = [vbias(out3T, kv, f"b{i}") for i, kv in enumerate((Kv_i, Kv_f, Kv_l))]
    emit_chunk()
    out1, out1T = attend_out(mixTs[0], h1T, "t1")
    emit_chunk()
    vbs_both = (vbA, vbB)

    # t1_out = out1 @ W_rel.T + b_rel -> out[:, 0:R]
    t1ps = psm.tile([BC, R], F32, name="t1_ps", tag="ps")
    for ch in range(2):
        nc.tensor.matmul(t1ps[:], out1T[:, ch, :], W_relT[:, ch, :],
                         start=(ch == 0), stop=False)
    nc.tensor.matmul(t1ps[:], ones_bf[:], b_rel[:], start=False, stop=True)
    t1sb = ep.tile([BC, R], F32, name="t1sb")
    nc.scalar.copy(t1sb[:], t1ps[:])
    dma(out=out_ap[:, 0:R], in_=t1sb[:])



    # ---- relu + ent heads (conv stages produced earlier) ----
    k = 0
    for b in range(BC):
        for j in range(NCH):
            if k % 4 == 0:
                emit_chunk()            # spread remaining chunks evenly
            k += 1
            cps = [stages[(b, j, half)] for half in range(2)]
            ent_ps = pcv.tile([4, 512], F32, name="ent_ps", tag="conv_ps",
                              padded_shape=[128, 512])
            for v in range(2):
                vbs = vbs_both[v]
                for half in range(2):
                    r = rp.tile([P, 512], BF16, name="relu")
                    nc.vector.tensor_scalar(r[:], cps[half][:],
                                            vbs[0][:, half, b:b + 1], 0.0,
                                            op0=mybir.AluOpType.add,
                                            op1=mybir.AluOpType.max)
                    if j == 0:
                        nc.vector.tensor_scalar(r[:, 0:1], cps[half][:, 0:1],
                                                vbs[1][:, half, b:b + 1], 0.0,
                                                op0=mybir.AluOpType.add,
                                                op1=mybir.AluOpType.max)
                    if j == NCH - 1:
                        nc.vector.tensor_scalar(r[:, 511:512],
                                                cps[half][:, 511:512],
                                                vbs[2][:, half, b:b + 1], 0.0,
                                                op0=mybir.AluOpType.add,
                                                op1=mybir.AluOpType.max)
                    nc.tensor.matmul(ent_ps[:], WentP[:, v, half, :],
                                     r[:],
                                     start=(v == 0 and half == 0),
                                     stop=(v == 1 and half == 1))
            esb = ep.tile([4, 512], F32, name="esb", bufs=4)
            nc.scalar.copy(esb[:], ent_ps[:])
            dma(out=outv[b, :, j * 512:(j + 1) * 512], in_=esb[:])


def build_nc():
    nc = bacc.Bacc("TRN2", target_bir_lowering=False, debug=False)
    io = {}

    def din(name, shape, dt):
        io[name] = nc.dram_tensor(name, shape, dt, kind="ExternalInput")

    din("ehl", [BC, 128, 2, 2, S], F8)
    din("enc_sc", [BC, 128, 2, 16, E], F8)
    din("wblob", [128, WTOT], BF16)
    din("wblob8", [128, W8TOT], F8)
    din("wblob85", [128, W8TOT], mybir.dt.float8e5)
    din("c0", [BC, E], F32)
    io["out"] = nc.dram_tensor("out", [BC, R + 4 * S], F32, kind="ExternalOutput")

    with ExitStack() as ctx:
        t = ctx.enter_context(tile.TileContext(nc))
        _emit(ctx, t, nc, io)
    nc.compile()
    return nc


def _pack2(w):  # [256, N] fp32 -> [128, 2, N]
    return np.ascontiguousarray(w.reshape(2, 128, -1).transpose(1, 0, 2))


def prepare_in_maps(inputs):
    bf = ml_dtypes.bfloat16
    f8 = ml_dtypes.float8_e4m3
    enc = np.asarray(inputs["encoder_o"], np.float32)

    def q8(x):
        return x.astype(f8)

    # fp8 enc splits, [B, 128, hl, t, S] with [b,p,hl,t,s] = val(enc[b,s,t*128+p])
    encT = enc.transpose(0, 2, 1)                  # [B, E, S]
    ehi = q8(encT)
    elo = q8(encT - ehi.astype(np.float32))
    # [B, E, S] -> [B, 128, 2, S]
    def esplit(a):
        return np.ascontiguousarray(a.reshape(B, 2, 128, S).transpose(0, 2, 1, 3))
    ehl = np.ascontiguousarray(
        np.stack([esplit(ehi), esplit(elo)], axis=2))   # [B,128,2,2,S]
    eS = enc.reshape(B, 16, 128, E).transpose(0, 2, 1, 3)   # [B,128,16,E]
    eSh = q8(eS)
    eSl = q8(eS - eSh.astype(np.float32))
    enc_sc = np.ascontiguousarray(np.stack([eSh, eSl], axis=2))  # [B,128,2,16,E]

    W_ih = np.asarray(inputs["W_ih"], np.float32)
    W_hh = np.asarray(inputs["W_hh"], np.float32)
    W_attn = np.asarray(inputs["W_attn"], np.float32)
    kern = np.asarray(inputs["W_conv"], np.float32).transpose(2, 1, 0)  # [3,2E,E]
    Kenc_ = kern[:, :E, :]
    Kv = kern[:, E:, :]
    Kv_i, Kv_f, Kv_l = Kv.sum(0), Kv[1] + Kv[2], Kv[0] + Kv[1]

    # conv fp8 weights, DoubleRow layout [128, w, co_half, ci_t, 128m]
    f85 = ml_dtypes.float8_e5m2
    K8 = q8(Kenc_)
    dK5 = (Kenc_ - K8.astype(np.float32)).astype(f85)
    def kpack(K):   # [3, 2E, E] fp8 -> [128, 3, 2, 2, 128]
        return np.ascontiguousarray(
            K.reshape(3, 2, 128, 2, 128).transpose(2, 0, 3, 1, 4))
    K8p, dK5p = kpack(K8), kpack(dK5)

    We = np.stack([np.asarray(inputs["W_ent1"])[0],
                   np.asarray(inputs["W_ent2"])[0]], 1)   # [E, 2]
    # WentP [128, v2, half2, 4]: [p, v, h, c] = We[h*128+p, c-2v] for c in {2v,2v+1}
    WentP = np.zeros((128, 2, 2, 4), np.float32)
    for v in range(2):
        for h in range(2):
            for head in range(2):
                WentP[:, v, h, 2 * v + head] = We[h * 128:(h + 1) * 128, head]

    x1 = np.broadcast_to(np.asarray(inputs["sos_emb"])[0], (B, E))
    x2 = np.asarray(inputs["rel_emb"])[np.asarray(inputs["r_in"]).astype(np.int64)]
    idx = np.arange(B)
    k1 = np.asarray(inputs["k1"])[:, 0].astype(np.int64)
    k2 = np.asarray(inputs["k2"])[:, 0].astype(np.int64)
    x3 = enc[idx, k1] + enc[idx, k2]
    X = np.stack([x1, x2, x3], 0).astype(np.float32)      # [3,B,E]
    h0 = np.asarray(inputs["h0"], np.float32)[0]
    c0 = np.asarray(inputs["c0"], np.float32)

    wsh = np.zeros((128, WTOT), np.float32)

    def put(name, arr):
        o, n = WOFF[name]
        wsh[:, o:o + n] = arr.reshape(128, n)

    def putrow(name, vec):
        o, n = WOFF[name]
        wsh[0, o:o + n] = vec.ravel()

    put("W_ihT", _pack2(W_ih.T))
    put("W_hhT", _pack2(W_hh.T))
    put("Wa_mT", _pack2(W_attn[:, :E].T))
    put("Wa_qT", _pack2(W_attn[:, E:].T))
    put("Kv_i", _pack2(Kv_i))
    put("Kv_f", _pack2(Kv_f))
    put("Kv_l", _pack2(Kv_l))
    put("W_relT", _pack2(np.asarray(inputs["W_rel"], np.float32).T))
    put("WentP", WentP)
    putrow("bias_g", np.asarray(inputs["b_ih"], np.float32)
           + np.asarray(inputs["b_hh"], np.float32))
    putrow("b_attn", np.asarray(inputs["b_attn"], np.float32))
    putrow("b_conv", np.asarray(inputs["b_conv"], np.float32))
    putrow("b_rel", np.asarray(inputs["b_rel"], np.float32))

    w8 = K8p.reshape(128, W8TOT)
    w85 = dK5p.reshape(128, W8TOT)

    in_maps = []
    for c in range(NCORES):
        sl = slice(c * BC, (c + 1) * BC)
        w = wsh.copy()
        xs = X[:, sl]                                      # [3,BC,E]
        xo, xn = WOFF["xT"]
        w[:, xo:xo + xn] = xs.transpose(2, 0, 1).reshape(
            2, 128, 3, BC).transpose(1, 2, 0, 3).reshape(128, xn)
        ho, hn = WOFF["h0T"]
        w[:, ho:ho + hn] = h0[sl].T.reshape(2, 128, BC).transpose(
            1, 0, 2).reshape(128, hn)
        m = {
            "ehl": np.ascontiguousarray(ehl[sl]),
            "enc_sc": np.ascontiguousarray(enc_sc[sl]),
            "wblob": w.astype(bf),
            "wblob8": w8.astype(f8),
            "wblob85": w85.astype(f85),
            "c0": np.ascontiguousarray(c0[0, sl]) if c0.ndim == 3
            else np.ascontiguousarray(c0[sl]),
        }
        in_maps.append(m)
    return in_maps


_NC_CACHE = {}


def get_nc():
    if "nc" not in _NC_CACHE:
        _NC_CACHE["nc"] = build_nc()
    return _NC_CACHE["nc"]


def kernel(**inputs) -> np.ndarray:
    nc = get_nc()
    in_maps = prepare_in_maps(inputs)
    res = run_bass_kernel_spmd(nc, in_maps, core_ids=list(range(NCORES)))
    out = np.concatenate([r["out"] for r in res.results], 0).astype(np.float32)
    # entity-head bias applied here (host): columns R.. hold e1a,e2a,e1b,e2b
    b1 = np.float32(np.asarray(inputs["b_ent1"]).ravel()[0])
    b2 = np.float32(np.asarray(inputs["b_ent2"]).ravel()[0])
    for vh, bb in enumerate((b1, b2, b1, b2)):
        out[:, R + vh * S:R + (vh + 1) * S] += bb
    return out


if __name__ == "__main__":
    import jax
    import reference as refmod
    with jax.default_device(jax.devices("cpu")[0]):
        inputs = {k: np.asarray(v) for k, v in refmod.setup_inputs().items()}
        expected = np.asarray(refmod.reference(**inputs))
    actual = kernel(**inputs)
    err = np.abs(actual - expected)
    print("max abs err:", err.max(), "rel:", err.max() / np.abs(expected).max())
